# revision 47
# baseline (speedup 1.0000x reference)
"""Multi-head attention (B=2, L=2048, H=1024, NH=16) on 8 TRN2 NeuronCores.

Sharding: data-parallel over batch (2) x tensor-parallel over heads (4 groups
of 4 heads).  core = b*4 + g handles batch b, heads [4g, 4g+4).  Wq/Wk/Wv are
split column-wise, Wo row-wise; each core produces a partial [L, H] output
that the host sums per batch (the row-parallel all-reduce done host-side).

Device math (per core), all matmuls bf16 inputs / fp32 PSUM accumulation:
  QT = (Wq*0.125)^T x^T          [256, 2048]  (softmax scale folded into Wq)
  KT = Wk^T y^T                  [256, 2048]
  V  = y Wv                      [2048, 256] stored as V_aug [lk, 4*(64+1)]
                                 with a ones column per head
  attention runs in HEAD-PAIR slots: heads (2j, 2j+1) live at partitions
  0:64 / 64:128 of QT/KT, so their S^T matmuls (contraction d=64) carry
  tile_position row groups (0,0)/(64,0) and execute CONCURRENTLY on the two
  row-halves of the PE array (~386ns for the pair vs 2x379 serial).
  Each slot (lkt, sl):
    S^T_a -> psS[:, 0:512], S^T_b -> psS[:, 512:1024]   (one paired burst)
    pt = exp(psS[:, 0:1024])     one FD=1024 ACTIVATE for both heads
    O^T_h[65, 512] += V_aug_h^T pt[:, po*512:]          (row 64 = softmax sums)
  128 slots in 4 segments (pair0-c0 lkt-major, then pair0-c1 / pair1-c0 /
  pair1-c1 sl-major).  PSUM: psS double-buffered [128,1024] (4 banks) +
  four single-bank [128,512] O accumulators keyed by (sl, head-parity).
  sl-major segments close their sl0 accumulators mid-segment, which (a)
  staggers the psO handover so segment boundaries don't stall, and (b)
  frees two banks that in-stream hook work (projections, output-projection
  pieces) borrows for its psum -- keeping the psS ring free so the S-pair /
  exp cadence never hiccups.  The last normalize chain of each segment is
  deferred into the next segment's slot 2 to keep the boundary DVE queue
  short.  V and the Q/K ct1 projections ride as hook thunks scheduled into
  specific slots (V's second half always lands in the PE FIFO before the
  O matmul that consumes it).  Dummy matmuls on scratch bridge the
  DMA-bound startup and the tail normalize window so the PE HAM clock-gate
  stays at 2.4 GHz.  Output partials are staged and DMA'd as bf16; the
  host accumulates the 4 head-group partials per batch in fp32.
"""

import numpy as np
import ml_dtypes

B, L, H, NH, D = 2, 2048, 1024, 16, 64
GP = 4            # head-groups (tensor-parallel factor)
CH = H // GP      # 256 local projection cols per core
HL = NH // GP     # 4 local heads
LQ = 1024         # lq chunk size
NLQ = L // LQ
NKT = L // 128    # 16 lk tiles
BF16 = ml_dtypes.bfloat16

_CACHE = {}


def _build():
    import concourse.mybir as mybir
    import concourse.tile as tile
    from concourse import bacc

    dt = mybir.dt
    f32, bf16 = dt.float32, dt.bfloat16
    Exp = mybir.ActivationFunctionType.Exp

    nc = bacc.Bacc("TRN2", target_bir_lowering=False, debug=False)
    # all inputs host-packed partition-major so each DMA is 128 long
    # contiguous runs (SP descriptor generation is the startup bottleneck)
    xT = nc.declare_dram_parameter("xT", [128, NLQ, 2, 8, 512], bf16,
                                   isOutput=False)
    yT = nc.declare_dram_parameter("yT", [128, NLQ, 2, 8, 512], bf16,
                                   isOutput=False)
    wq = nc.declare_dram_parameter("wq", [128, 8, CH], bf16, isOutput=False)
    wk = nc.declare_dram_parameter("wk", [128, 8, CH], bf16, isOutput=False)
    wv = nc.declare_dram_parameter("wv", [128, 8, CH], bf16, isOutput=False)
    wo = nc.declare_dram_parameter("wo", [128, 2, H], bf16, isOutput=False)
    # bf16 partial output: host accumulates the 4 head-group partials in
    # fp32, so the bf16 rounding (~0.4% per partial) is well inside budget
    out = nc.declare_dram_parameter("out", [L, H], bf16, isOutput=True)

    with tile.TileContext(nc) as tc:
        with (
            tc.tile_pool(name="w", bufs=1) as wpool,
            tc.tile_pool(name="acts", bufs=1) as apool,
            tc.tile_pool(name="psS", bufs=2, space="PSUM") as psS,
            tc.tile_pool(name="psO", bufs=1, space="PSUM") as psO,
            tc.tile_pool(name="pt", bufs=6) as ptpool,
            tc.tile_pool(name="oT", bufs=2) as otpool,
            tc.tile_pool(name="sm", bufs=3) as smpool,
            tc.tile_pool(name="osb", bufs=4) as opool,
        ):
            # prefetch the exp activation table while input DMAs run
            dummy = smpool.tile([1, 8], f32, tag="dummy")
            nc.vector.memset(dummy, 0.0)
            nc.scalar.activation(dummy, dummy, Exp)

            # PE warm-up fodder: matmuls on a memset scratch keep the HAM
            # activity window busy through the DMA-bound startup so the real
            # stream starts (and stays) at 2.4 GHz.
            scratch = apool.tile([128, 512], bf16, tag="scratch")
            nc.vector.memset(scratch, 0.0)

            # ---- input DMAs ordered by first use: weights, then the
            # activation chunks the startup projections + first slots need -
            wk_sb = wpool.tile([128, 8, CH], bf16, tag="wk")
            nc.sync.dma_start(wk_sb, wk[:, :, :])
            wq_sb = wpool.tile([128, 8, CH], bf16, tag="wq")
            nc.sync.dma_start(wq_sb, wq[:, :, :])
            yT_sb = apool.tile([128, NLQ, 2, 8, 512], bf16, tag="yT")
            xT_sb = apool.tile([128, NLQ, 2, 8, 512], bf16, tag="xT")
            # y00/x00 land in two half-MB pieces each: the startup K/Q
            # projections start on the first half (per-subtile deps), without
            # paying per-ht Sync dispatch overhead (~0.7us per dma_start).
            # wv comes after x00: V's first consumer runs ~10 slots into the
            # stream, while exp(0) gates on K1(y00) -> Q1(x00) directly.
            nc.sync.dma_start(yT_sb[:, 0, 0, 0:4], yT[:, 0, 0, 0:4])
            nc.sync.dma_start(yT_sb[:, 0, 0, 4:8], yT[:, 0, 0, 4:8])
            nc.sync.dma_start(xT_sb[:, 0, 0, 0:4], xT[:, 0, 0, 0:4])
            nc.sync.dma_start(xT_sb[:, 0, 0, 4:8], xT[:, 0, 0, 4:8])
            wv_sb = wpool.tile([128, 8, CH], bf16, tag="wv")
            nc.sync.dma_start(wv_sb, wv[:, :, :])
            nc.sync.dma_start(xT_sb[:, 0, 1], xT[:, 0, 1])
            nc.sync.dma_start(yT_sb[:, 0, 1], yT[:, 0, 1])
            nc.sync.dma_start(yT_sb[:, 1, 0], yT[:, 1, 0])
            nc.sync.dma_start(yT_sb[:, 1, 1], yT[:, 1, 1])
            nc.sync.dma_start(xT_sb[:, 1, 0], xT[:, 1, 0])
            nc.sync.dma_start(xT_sb[:, 1, 1], xT[:, 1, 1])
            # (y00/x00/x01 first: the startup K/Q projections gate exp(0))
            wo_sb = wpool.tile([128, 2, H], bf16, tag="wo")
            nc.sync.dma_start(wo_sb, wo[:, :, :])

            # warm-up matmuls (no data deps beyond the scratch memset):
            # enough to flip the HAM clock-gate, short enough to drain
            # before the first input chunk lands even on a fast DMA run
            for _w in range(2):
                wps = psS.tile([128, LQ], f32, tag="psS", name=f"warm{_w}")
                for _i in range(8):
                    nc.tensor.matmul(
                        wps[:, 0:512],
                        lhsT=scratch[:, 0:128], rhs=scratch[:, 0:512],
                        start=True, stop=True,
                    )

            qT_sb = apool.tile([128, 2, L], bf16, tag="qT")
            kT_sb = apool.tile([128, 2, L], bf16, tag="kT")
            vaug_sb = apool.tile([128, NKT, HL * 65], bf16, tag="vaug")

            def proj_group(w_sb, act_sb, dst, ct, lh, sl):
                # dst[:, ct, lh*LQ+sl*512 : +512] via one 8-matmul psum group
                ps = psS.tile([128, LQ], f32, tag="psS", name="projps")
                off = lh * LQ + sl * 512
                for ht in range(8):
                    nc.tensor.matmul(
                        ps[:, 0:512],
                        lhsT=w_sb[:, ht, ct * 128:(ct + 1) * 128],
                        rhs=act_sb[:, lh, sl, ht, :],
                        start=(ht == 0), stop=(ht == 7),
                    )
                nc.vector.tensor_copy(dst[:, ct, off:off + 512], ps[:, 0:512])

            def pj2(w_sb, act_sb, dst, ct, lh, sl, ps_tag=None):
                # one projection psum group split into two 4-matmul thunks.
                # ps_tag borrows an idle psO bank (sl-major segments always
                # have the opposite phase's banks free) instead of inserting
                # into the psS ring, which would stall the S-pair cadence.
                cell = {}

                def half(r):
                    def thunk():
                        if r == 0:
                            if ps_tag is None:
                                cell["ps"] = psS.tile(
                                    [128, LQ], f32, tag="psS",
                                    name=f"pjps{ct}_{lh}_{sl}_{id(w_sb) % 97}")
                            else:
                                cell["ps"] = psO.tile(
                                    [128, 512], f32, tag=ps_tag,
                                    name=f"pjps{ct}_{lh}_{sl}_{id(w_sb) % 97}")
                        ps = cell["ps"]
                        for ht in range(4 * r, 4 * r + 4):
                            nc.tensor.matmul(
                                ps[:, 0:512],
                                lhsT=w_sb[:, ht, ct * 128:(ct + 1) * 128],
                                rhs=act_sb[:, lh, sl, ht, :],
                                start=(ht == 0), stop=(ht == 7),
                            )
                        if r == 1:
                            nc.vector.tensor_copy(
                                dst[:, ct, lh * LQ + sl * 512:
                                    lh * LQ + (sl + 1) * 512], ps[:, 0:512])
                    return thunk
                return half(0), half(1)

            def v2(lkt):
                # one lk tile of V_aug[lk, 4*(64+1)] bf16, split in 2 thunks
                cell = {}

                def half(r):
                    def thunk():
                        if r == 0:
                            cell["ps"] = psS.tile(
                                [128, LQ], f32, tag="psS", name=f"vps{lkt}")
                        psv = cell["ps"]
                        for ht in range(4 * r, 4 * r + 4):
                            nc.tensor.matmul(
                                psv[:, :CH],
                                lhsT=yT_sb[:, lkt // 8, (lkt % 8) // 4, ht,
                                           (lkt % 4) * 128:(lkt % 4 + 1) * 128],
                                rhs=wv_sb[:, ht, :],
                                start=(ht == 0), stop=(ht == 7),
                            )
                        if r == 1:
                            vh = vaug_sb[:, lkt, :].rearrange(
                                "p (h e) -> p h e", h=HL)
                            nc.vector.tensor_copy(
                                vh[:, :, 0:64],
                                psv[:, :CH].rearrange("p (h e) -> p h e", h=HL))
                            nc.vector.memset(vh[:, :, 64], 1.0)
                    return thunk
                return half(0), half(1)

            def v_full(lkt):
                a, b = v2(lkt)
                a()
                b()

            oT = [otpool.tile([128, 2, LQ], bf16, tag="oT", name=f"oT{i}")
                  for i in range(NLQ)]

            def emit_S_pair(ci, ct2, sl, lkt):
                # both heads of the pair in one psS tile: po=0 -> cols 0:512,
                # po=1 -> cols 512:1024.  The two matmuls carry row groups
                # (0,0) and (64,0) and execute concurrently on the PE array.
                ps = psS.tile([128, LQ], f32, tag="psS", name="psSp")
                for po in range(2):
                    nc.tensor.matmul(
                        ps[:, po * 512:(po + 1) * 512],
                        lhsT=kT_sb[64 * po:64 * po + 64, ct2,
                                   lkt * 128:(lkt + 1) * 128],
                        rhs=qT_sb[64 * po:64 * po + 64, ct2,
                                  ci * LQ + sl * 512:ci * LQ + (sl + 1) * 512],
                        start=True, stop=True,
                    )
                return ps

            def normalize(ps_t, ci, ct2, po, sl, act_sums=False):
                sums = smpool.tile([1, 512], f32, tag="sums")
                if act_sums:
                    nc.scalar.copy(sums, ps_t[64:65, 0:512])
                else:
                    nc.vector.tensor_copy(sums, ps_t[64:65, 0:512])
                recip = smpool.tile([1, 512], f32, tag="recip")
                nc.vector.reciprocal_approx_fast(recip, sums)
                bcast = smpool.tile([64, 512], f32, tag="bcast")
                nc.gpsimd.partition_broadcast(bcast, recip)
                nc.vector.tensor_mul(
                    oT[ci][64 * po:64 * po + 64, ct2,
                           sl * 512:(sl + 1) * 512], ps_t[0:64, 0:512], bcast)

            def normalize_final_pair(ps_a, ps_b, ci, ct2, sl):
                # both heads' chains fused: one recip + ONE gpsimd
                # broadcast (avoids the ~1.2us inter-op GpSimd DRAIN on the
                # tail critical path); sums copies ride the idle ScalarE
                sums = smpool.tile([1, 1024], f32, tag="sumsP", bufs=1)
                nc.scalar.copy(sums[:, 0:512], ps_a[64:65, 0:512])
                nc.scalar.copy(sums[:, 512:1024], ps_b[64:65, 0:512])
                recip = smpool.tile([1, 1024], f32, tag="recipP", bufs=1)
                nc.vector.reciprocal_approx_fast(recip, sums)
                bcast = smpool.tile([64, 1024], f32, tag="bcastP", bufs=1)
                nc.gpsimd.partition_broadcast(bcast, recip)
                for po, pst in ((0, ps_a), (1, ps_b)):
                    nc.vector.tensor_mul(
                        oT[ci][64 * po:64 * po + 64, ct2,
                               sl * 512:(sl + 1) * 512],
                        pst[0:64, 0:512], bcast[:, po * 512:(po + 1) * 512])

            def s3_piece(ci, mt, act_copy=False, ps_tags=None):
                # out rows [ci*LQ + mt*128 : +128], full H width.  In-stream
                # pieces borrow the opposite sl-phase's two idle psO banks
                # (ps_tags) so the psS ring's S-pair cadence is untouched;
                # tail pieces use the then-idle psS ring.
                osb = opool.tile([128, LQ], bf16, tag="osb")
                if ps_tags is None:
                    pso = psS.tile([128, LQ], f32, tag="psS",
                                   name=f"s3ps{ci}_{mt}")
                    halves = [pso[:, 0:512], pso[:, 512:1024]]
                else:
                    halves = [psO.tile([128, 512], f32, tag=t,
                                       name=f"s3ps{ci}_{mt}_{nt}")
                              for nt, t in enumerate(ps_tags)]
                for nt in range(2):
                    for kt in range(2):
                        nc.tensor.matmul(
                            halves[nt],
                            lhsT=oT[ci][:, kt, mt * 128:(mt + 1) * 128],
                            rhs=wo_sb[:, kt, nt * 512:(nt + 1) * 512],
                            start=(kt == 0), stop=(kt == 1),
                        )
                if ps_tags is None:
                    if act_copy:
                        nc.scalar.copy(osb, pso)
                    else:
                        nc.vector.tensor_copy(osb, pso)
                else:
                    for nt in range(2):
                        nc.vector.tensor_copy(
                            osb[:, nt * 512:(nt + 1) * 512], halves[nt])
                nc.sync.dma_start(
                    out[ci * LQ + mt * 128:ci * LQ + (mt + 1) * 128, :], osb)

            pipe = {}

            # ---- startup: only what the first slots strictly need; the
            # first S pair goes into the PE FIFO straight after the Q
            # projection it reads so exp(0) isn't queued behind V/Q-sl1 ----
            proj_group(wk_sb, yT_sb, kT_sb, 0, 0, 0)   # K ct0 lk 0:512
            proj_group(wq_sb, xT_sb, qT_sb, 0, 0, 0)   # Q ct0 lq 0:512
            pipe["ps"] = emit_S_pair(0, 0, 0, 0)
            proj_group(wq_sb, xT_sb, qT_sb, 0, 0, 1)   # Q ct0 lq 512:1024
            # V(0) first half here; its second half is seg1's slot-0 hook
            # so it lands in the PE FIFO before the O matmul that reads it
            v0a, v0b = v2(0)
            v0a()

            def seg(ci, ct2, sched, nxt, sl_major=False):
                # slot k -> (lkt, sl).  lkt-major relaxes the K-projection
                # and V deadlines (default); sl-major closes the sl0 O
                # accumulators mid-segment so the chunk's output projection
                # can start inside the stream (used for the last segment).
                if sl_major:
                    order = [(k % NKT, k // NKT) for k in range(32)]
                else:
                    order = [(k // 2, k % 2) for k in range(32)]
                ps_t = {}

                def get_ps(sl, po):
                    # claim the accumulator at first use: in sl-major order
                    # the sl1 tiles are claimed only at slot 16, AFTER any
                    # hook that borrowed those banks during the sl0 phase
                    if (sl, po) not in ps_t:
                        ps_t[(sl, po)] = psO.tile(
                            [128, 512], f32, tag=f"psO{sl}{po}",
                            name=f"psO{ci}{ct2}{sl}{po}")
                    return ps_t[(sl, po)]

                if not sl_major:
                    for sl in range(2):
                        for po in range(2):
                            get_ps(sl, po)
                for k in range(32):
                    lkt, sl = order[k]
                    ps = pipe.pop("ps")
                    pt = ptpool.tile([128, LQ], bf16, tag="pt")
                    nc.scalar.activation(pt, ps, Exp)
                    if k + 1 < 32:
                        nl, nsl = order[k + 1]
                        pipe["ps"] = emit_S_pair(ci, ct2, nsl, nl)
                    elif nxt is not None:
                        pipe["ps"] = emit_S_pair(nxt[0], nxt[1], 0, 0)
                    if k == 2 and "defer" in pipe:
                        pipe.pop("defer")()
                    for job in sched.get(k, ()):
                        job()
                    for po in range(2):
                        h = 2 * ct2 + po
                        nc.tensor.matmul(
                            get_ps(sl, po)[0:65, 0:512],
                            lhsT=vaug_sb[:, lkt, h * 65:(h + 1) * 65],
                            rhs=pt[:, po * 512:(po + 1) * 512],
                            start=(lkt == 0), stop=(lkt == NKT - 1),
                        )
                    if lkt == NKT - 1:
                        if sl == 1 and nxt is None:
                            # final segment: fused pair chain on the tail
                            # critical path
                            normalize_final_pair(
                                ps_t[(1, 0)], ps_t[(1, 1)], ci, ct2, 1)
                            continue
                        # the very last chain of a segment is deferred into
                        # the next segment's slot 2: it shortens the DVE
                        # queue at the boundary, where the next segment's
                        # first borrowed-psum hooks wait on chain completion
                        for po in range(2):
                            if sl == 1 and po == 1 and nxt is not None:
                                t = ps_t[(sl, po)]
                                pipe["defer"] = (
                                    lambda t=t, a=ci, b=ct2, c=po, d=sl:
                                    normalize(t, a, b, c, d))
                            else:
                                normalize(ps_t[(sl, po)], ci, ct2, po, sl)

            def sched_pairs(pairs):
                # pairs: list of ((thunk_a, thunk_b), (slot_a, slot_b))
                sched = {}
                for (a, b), (sa, sb_) in pairs:
                    sched.setdefault(sa, []).append(a)
                    sched.setdefault(sb_, []).append(b)
                return sched

            # SEG1 (pair0, chunk0), lkt-major: all of V + remaining K ct0 +
            # Q ct0 lh1.  V halves 1/slot; K/Q halves overlay (those slots
            # run PE-paced).  Hook psum comes from the psS ring here (no
            # idle psO banks in lkt-major order).
            seg1 = sched_pairs(
                [((v0b, lambda: None), (0, 1))] +
                [(v2(j), (2 * j - 2, 2 * j - 1)) for j in range(1, 16)] +
                [(pj2(wk_sb, yT_sb, kT_sb, 0, 0, 1), (3, 5)),
                 (pj2(wk_sb, yT_sb, kT_sb, 0, 1, 0), (9, 11)),
                 (pj2(wk_sb, yT_sb, kT_sb, 0, 1, 1), (15, 17)),
                 (pj2(wq_sb, xT_sb, qT_sb, 0, 1, 0), (24, 26))])
            # SEG2 (pair0, chunk1), sl-major: Q ct0 lh1 sl1 (needed by this
            # segment's own sl1 phase), K ct1, Q ct1 lh0 — all borrowing
            # the opposite phase's idle psO banks for projection psum
            seg2 = sched_pairs(
                [(pj2(wk_sb, yT_sb, kT_sb, 1, 0, 0, "psO10"), (6, 8)),
                 (pj2(wq_sb, xT_sb, qT_sb, 0, 1, 1, "psO11"), (7, 9)),
                 (pj2(wk_sb, yT_sb, kT_sb, 1, 0, 1, "psO10"), (10, 12)),
                 (pj2(wk_sb, yT_sb, kT_sb, 1, 1, 0, "psO11"), (11, 13)),
                 (pj2(wk_sb, yT_sb, kT_sb, 1, 1, 1, "psO00"), (18, 20)),
                 (pj2(wq_sb, xT_sb, qT_sb, 1, 0, 0, "psO01"), (22, 24)),
                 (pj2(wq_sb, xT_sb, qT_sb, 1, 0, 1, "psO00"), (26, 28))])
            # SEG3 (pair1, chunk0), sl-major: Q ct1 lh1, then the sl0 half
            # of chunk-0's output projection (this segment's own sl0
            # normalize completes mid-segment)
            seg3 = sched_pairs(
                [(pj2(wq_sb, xT_sb, qT_sb, 1, 1, 0, "psO10"), (6, 8)),
                 (pj2(wq_sb, xT_sb, qT_sb, 1, 1, 1, "psO11"), (10, 12))])
            for mt, s in zip(range(4), (20, 22, 24, 26)):
                seg3[s] = [(lambda mt=mt: s3_piece(
                    0, mt, ps_tags=("psO00", "psO01")))]
            # SEG4 (pair1, chunk1), sl-major: rest of chunk-0's output
            # projection + the sl0 half of chunk-1's
            seg4 = {}
            for mt, s in zip(range(4, 8), (6, 8, 10, 12)):
                seg4[s] = [(lambda mt=mt: s3_piece(
                    0, mt, ps_tags=("psO10", "psO11")))]
            for mt, s in zip(range(4), (20, 22, 24, 26)):
                seg4[s] = [(lambda mt=mt: s3_piece(
                    1, mt, ps_tags=("psO00", "psO01")))]

            seg(0, 0, seg1, nxt=(1, 0))
            seg(1, 0, seg2, nxt=(0, 1), sl_major=True)
            seg(0, 1, seg3, nxt=(1, 1), sl_major=True)
            seg(1, 1, seg4, nxt=None, sl_major=True)
            # warm bridge: dummy matmuls keep the HAM clock-gate open while
            # the final sl1 normalize chains run on DVE/GpSimd, so the tail
            # output-projection matmuls execute at 2.4 GHz
            wps = psS.tile([128, LQ], f32, tag="psS", name="warmtail")
            for _i in range(10):
                nc.tensor.matmul(
                    wps[:, 0:512],
                    lhsT=scratch[:, 0:128], rhs=scratch[:, 0:512],
                    start=True, stop=True,
                )
            # tail: remaining chunk-1 output projection.  Both ScalarE and
            # DVE are idle once the final chains drain -> alternate the
            # copies so they pipeline two-wide behind the matmuls.
            for mt in range(4, LQ // 128):
                s3_piece(1, mt, act_copy=bool(mt % 2 == 0))
    nc.compile()
    return nc


def _get_nc():
    if "nc" not in _CACHE:
        _CACHE["nc"] = _build()
    return _CACHE["nc"]


def _pack_pm(a, t):
    # [t*128, N] -> [128, t, N] partition-major
    return a.reshape(t, 128, -1).transpose(1, 0, 2)


def _pack_act(a):
    # x[b] [L, H] -> xT packed [128, NLQ(lh), 2(sl), 8(t), 512] bf16
    v = _pack_pm(np.ascontiguousarray(a.T), 8)          # [128, 8, L]
    v = v.reshape(128, 8, NLQ, 2, 512).transpose(0, 2, 3, 1, 4)
    return np.ascontiguousarray(v).astype(BF16)


def _in_maps(x, y, Wq, Wk, Wv, Wo):
    maps = []
    for core in range(8):
        b, g = core // GP, core % GP
        cs = slice(g * CH, (g + 1) * CH)
        maps.append({
            "xT": _pack_act(x[b]),
            "yT": _pack_act(y[b]),
            "wq": np.ascontiguousarray(
                _pack_pm(Wq[:, cs] * np.float32(0.125), 8)).astype(BF16),
            "wk": np.ascontiguousarray(_pack_pm(Wk[:, cs], 8)).astype(BF16),
            "wv": np.ascontiguousarray(_pack_pm(Wv[:, cs], 8)).astype(BF16),
            "wo": np.ascontiguousarray(_pack_pm(Wo[cs, :], 2)).astype(BF16),
        })
    return maps


def _install_ntff_hook():
    """Provide the antenv.axon_hooks shim missing from this container so
    run_bass_kernel_spmd(trace=True) can drive NTFF profiling via ctypes."""
    import sys
    import types
    try:
        from antenv.axon_hooks import get_axon_ntff_profile_hook  # noqa: F401
        return
    except ImportError:
        pass
    from trn_agent_boot.trn_boot import _ntff_profile_via_ctypes
    hook = _ntff_profile_via_ctypes("/opt/axon/libaxon_pjrt.so")
    mod = types.ModuleType("antenv.axon_hooks")
    mod.get_axon_ntff_profile_hook = lambda: hook
    mod.set_axon_ntff_profile_hook = lambda h: None
    sys.modules["antenv.axon_hooks"] = mod


def _run(inputs, trace=False):
    from concourse import bass_utils

    if trace:
        _install_ntff_hook()

    x, y, bias = inputs["x"], inputs["y"], inputs["bias"]
    if np.count_nonzero(np.asarray(bias)):
        raise NotImplementedError("nonzero attention bias not supported")
    nc = _get_nc()
    maps = _in_maps(np.asarray(x, np.float32), np.asarray(y, np.float32),
                    np.asarray(inputs["Wq"], np.float32),
                    np.asarray(inputs["Wk"], np.float32),
                    np.asarray(inputs["Wv"], np.float32),
                    np.asarray(inputs["Wo"], np.float32))
    res = bass_utils.run_bass_kernel_spmd(
        nc, maps, list(range(8)), trace=trace)
    out = np.zeros((B, L, H), np.float32)
    for core in range(8):
        out[core // GP] += np.asarray(res.results[core]["out"], np.float32)
    return out, res


def kernel(**inputs):
    out, _ = _run(inputs, trace=False)
    return out


# revision 49
# speedup vs baseline: 1.1827x; 1.1827x over previous
"""Multi-head attention (B=2, L=2048, H=1024, NH=16) on 8 TRN2 NeuronCores.

Sharding: data-parallel over batch (2) x tensor-parallel over heads (4 groups
of 4 heads).  core = b*4 + g handles batch b, heads [4g, 4g+4).  Wq/Wk/Wv are
split column-wise, Wo row-wise; each core produces a partial [L, H] output
that the host sums per batch (the row-parallel all-reduce done host-side).

Device math (per core), all matmuls bf16 inputs / fp32 PSUM accumulation:
  QT = (Wq*0.125)^T x^T          [256, 2048]  (softmax scale folded into Wq)
  KT = Wk^T y^T                  [256, 2048]
  V  = y Wv                      [2048, 256] stored as V_aug [lk, 4*(64+1)]
                                 with a ones column per head
  attention runs in HEAD-PAIR slots: heads (2j, 2j+1) live at partitions
  0:64 / 64:128 of QT/KT, so their S^T matmuls (contraction d=64) carry
  tile_position row groups (0,0)/(64,0) and execute CONCURRENTLY on the two
  row-halves of the PE array (~386ns for the pair vs 2x379 serial).
  Each slot (lkt, sl):
    S^T_a -> psS[:, 0:512], S^T_b -> psS[:, 512:1024]   (one paired burst)
    pt = exp(psS[:, 0:1024])     one FD=1024 ACTIVATE for both heads
    O^T_h[65, 512] += V_aug_h^T pt[:, po*512:]          (row 64 = softmax sums)
  128 slots in 4 segments (pair0-c0 lkt-major, then pair0-c1 / pair1-c0 /
  pair1-c1 sl-major).  PSUM: psS double-buffered [128,1024] (4 banks) +
  four single-bank [128,512] O accumulators keyed by (sl, head-parity).
  sl-major segments close their sl0 accumulators mid-segment, which (a)
  staggers the psO handover so segment boundaries don't stall, and (b)
  frees two banks that in-stream hook work (projections, output-projection
  pieces) borrows for its psum -- keeping the psS ring free so the S-pair /
  exp cadence never hiccups.  The last normalize chain of each segment is
  deferred into the next segment's slot 2 to keep the boundary DVE queue
  short.  V and the Q/K ct1 projections ride as hook thunks scheduled into
  specific slots (V's second half always lands in the PE FIFO before the
  O matmul that consumes it).  Dummy matmuls on scratch bridge the
  DMA-bound startup and the tail normalize window so the PE HAM clock-gate
  stays at 2.4 GHz.  Output partials are staged and DMA'd as bf16; the
  host accumulates the 4 head-group partials per batch in fp32.
"""

import numpy as np
import ml_dtypes

B, L, H, NH, D = 2, 2048, 1024, 16, 64
GP = 4            # head-groups (tensor-parallel factor)
CH = H // GP      # 256 local projection cols per core
HL = NH // GP     # 4 local heads
LQ = 1024         # lq chunk size
NLQ = L // LQ
NKT = L // 128    # 16 lk tiles
BF16 = ml_dtypes.bfloat16

_CACHE = {}


def _build():
    import concourse.mybir as mybir
    import concourse.tile as tile
    from concourse import bacc

    dt = mybir.dt
    f32, bf16 = dt.float32, dt.bfloat16
    Exp = mybir.ActivationFunctionType.Exp

    nc = bacc.Bacc("TRN2", target_bir_lowering=False, debug=False)
    # all inputs host-packed partition-major so each DMA is 128 long
    # contiguous runs (SP descriptor generation is the startup bottleneck)
    xT = nc.declare_dram_parameter("xT", [128, NLQ, 2, 8, 512], bf16,
                                   isOutput=False)
    yT = nc.declare_dram_parameter("yT", [128, NLQ, 2, 8, 512], bf16,
                                   isOutput=False)
    wq = nc.declare_dram_parameter("wq", [128, 8, CH], bf16, isOutput=False)
    wk = nc.declare_dram_parameter("wk", [128, 8, CH], bf16, isOutput=False)
    wv = nc.declare_dram_parameter("wv", [128, 8, CH], bf16, isOutput=False)
    wo = nc.declare_dram_parameter("wo", [128, 2, H], bf16, isOutput=False)
    # bf16 partial output: host accumulates the 4 head-group partials in
    # fp32, so the bf16 rounding (~0.4% per partial) is well inside budget
    out = nc.declare_dram_parameter("out", [L, H], bf16, isOutput=True)

    with tile.TileContext(nc) as tc:
        with (
            tc.tile_pool(name="w", bufs=1) as wpool,
            tc.tile_pool(name="acts", bufs=1) as apool,
            tc.tile_pool(name="psS", bufs=2, space="PSUM") as psS,
            tc.tile_pool(name="psO", bufs=1, space="PSUM") as psO,
            tc.tile_pool(name="pt", bufs=6) as ptpool,
            tc.tile_pool(name="oT", bufs=2) as otpool,
            tc.tile_pool(name="sm", bufs=3) as smpool,
            tc.tile_pool(name="osb", bufs=4) as opool,
        ):
            # prefetch the exp activation table while input DMAs run
            dummy = smpool.tile([1, 8], f32, tag="dummy")
            nc.vector.memset(dummy, 0.0)
            nc.scalar.activation(dummy, dummy, Exp)

            # PE warm-up fodder: matmuls on a memset scratch keep the HAM
            # activity window busy through the DMA-bound startup so the real
            # stream starts (and stays) at 2.4 GHz.
            scratch = apool.tile([128, 512], bf16, tag="scratch")
            nc.vector.memset(scratch, 0.0)

            # ---- input DMAs ordered by first use: weights, then the
            # activation chunks the startup projections + first slots need -
            wk_sb = wpool.tile([128, 8, CH], bf16, tag="wk")
            nc.sync.dma_start(wk_sb, wk[:, :, :])
            wq_sb = wpool.tile([128, 8, CH], bf16, tag="wq")
            nc.sync.dma_start(wq_sb, wq[:, :, :])
            yT_sb = apool.tile([128, NLQ, 2, 8, 512], bf16, tag="yT")
            xT_sb = apool.tile([128, NLQ, 2, 8, 512], bf16, tag="xT")
            # y00/x00 land in two half-MB pieces each: the startup K/Q
            # projections start on the first half (per-subtile deps), without
            # paying per-ht Sync dispatch overhead (~0.7us per dma_start).
            # wv comes after x00: V's first consumer runs ~10 slots into the
            # stream, while exp(0) gates on K1(y00) -> Q1(x00) directly.
            nc.sync.dma_start(yT_sb[:, 0, 0, 0:4], yT[:, 0, 0, 0:4])
            nc.sync.dma_start(yT_sb[:, 0, 0, 4:8], yT[:, 0, 0, 4:8])
            nc.sync.dma_start(xT_sb[:, 0, 0, 0:4], xT[:, 0, 0, 0:4])
            nc.sync.dma_start(xT_sb[:, 0, 0, 4:8], xT[:, 0, 0, 4:8])
            wv_sb = wpool.tile([128, 8, CH], bf16, tag="wv")
            nc.sync.dma_start(wv_sb, wv[:, :, :])
            nc.sync.dma_start(xT_sb[:, 0, 1], xT[:, 0, 1])
            nc.sync.dma_start(yT_sb[:, 0, 1], yT[:, 0, 1])
            nc.sync.dma_start(yT_sb[:, 1, 0], yT[:, 1, 0])
            nc.sync.dma_start(yT_sb[:, 1, 1], yT[:, 1, 1])
            nc.sync.dma_start(xT_sb[:, 1, 0], xT[:, 1, 0])
            nc.sync.dma_start(xT_sb[:, 1, 1], xT[:, 1, 1])
            # (y00/x00/x01 first: the startup K/Q projections gate exp(0))
            wo_sb = wpool.tile([128, 2, H], bf16, tag="wo")
            nc.sync.dma_start(wo_sb, wo[:, :, :])

            # warm-up matmuls (no data deps beyond the scratch memset):
            # enough to flip the HAM clock-gate, short enough to drain
            # before the first input chunk lands even on a fast DMA run
            for _w in range(2):
                wps = psS.tile([128, LQ], f32, tag="psS", name=f"warm{_w}")
                for _i in range(8):
                    nc.tensor.matmul(
                        wps[:, 0:512],
                        lhsT=scratch[:, 0:128], rhs=scratch[:, 0:512],
                        start=True, stop=True,
                    )

            qT_sb = apool.tile([128, 2, L], bf16, tag="qT")
            kT_sb = apool.tile([128, 2, L], bf16, tag="kT")
            vaug_sb = apool.tile([128, NKT, HL * 65], bf16, tag="vaug")

            def proj_group(w_sb, act_sb, dst, ct, lh, sl):
                # dst[:, ct, lh*LQ+sl*512 : +512] via one 8-matmul psum group
                ps = psS.tile([128, LQ], f32, tag="psS", name="projps")
                off = lh * LQ + sl * 512
                for ht in range(8):
                    nc.tensor.matmul(
                        ps[:, 0:512],
                        lhsT=w_sb[:, ht, ct * 128:(ct + 1) * 128],
                        rhs=act_sb[:, lh, sl, ht, :],
                        start=(ht == 0), stop=(ht == 7),
                    )
                nc.vector.tensor_copy(dst[:, ct, off:off + 512], ps[:, 0:512])

            def pj2(w_sb, act_sb, dst, ct, lh, sl, ps_tag=None):
                # one projection psum group split into two 4-matmul thunks.
                # ps_tag borrows an idle psO bank (sl-major segments always
                # have the opposite phase's banks free) instead of inserting
                # into the psS ring, which would stall the S-pair cadence.
                cell = {}

                def half(r):
                    def thunk():
                        if r == 0:
                            if ps_tag is None:
                                cell["ps"] = psS.tile(
                                    [128, LQ], f32, tag="psS",
                                    name=f"pjps{ct}_{lh}_{sl}_{id(w_sb) % 97}")
                            else:
                                cell["ps"] = psO.tile(
                                    [128, 512], f32, tag=ps_tag,
                                    name=f"pjps{ct}_{lh}_{sl}_{id(w_sb) % 97}")
                        ps = cell["ps"]
                        for ht in range(4 * r, 4 * r + 4):
                            nc.tensor.matmul(
                                ps[:, 0:512],
                                lhsT=w_sb[:, ht, ct * 128:(ct + 1) * 128],
                                rhs=act_sb[:, lh, sl, ht, :],
                                start=(ht == 0), stop=(ht == 7),
                            )
                        if r == 1:
                            nc.vector.tensor_copy(
                                dst[:, ct, lh * LQ + sl * 512:
                                    lh * LQ + (sl + 1) * 512], ps[:, 0:512])
                    return thunk
                return half(0), half(1)

            def v2(lkt):
                # one lk tile of V_aug[lk, 4*(64+1)] bf16, split in 2 thunks
                cell = {}

                def half(r):
                    def thunk():
                        if r == 0:
                            cell["ps"] = psS.tile(
                                [128, LQ], f32, tag="psS", name=f"vps{lkt}")
                        psv = cell["ps"]
                        for ht in range(4 * r, 4 * r + 4):
                            nc.tensor.matmul(
                                psv[:, :CH],
                                lhsT=yT_sb[:, lkt // 8, (lkt % 8) // 4, ht,
                                           (lkt % 4) * 128:(lkt % 4 + 1) * 128],
                                rhs=wv_sb[:, ht, :],
                                start=(ht == 0), stop=(ht == 7),
                            )
                        if r == 1:
                            vh = vaug_sb[:, lkt, :].rearrange(
                                "p (h e) -> p h e", h=HL)
                            nc.vector.tensor_copy(
                                vh[:, :, 0:64],
                                psv[:, :CH].rearrange("p (h e) -> p h e", h=HL))
                            nc.vector.memset(vh[:, :, 64], 1.0)
                    return thunk
                return half(0), half(1)

            def v_full(lkt):
                a, b = v2(lkt)
                a()
                b()

            oT = [otpool.tile([128, 2, LQ], bf16, tag="oT", name=f"oT{i}")
                  for i in range(NLQ)]

            def emit_S_pair(ci, ct2, sl, lkt):
                # both heads of the pair in one psS tile: po=0 -> cols 0:512,
                # po=1 -> cols 512:1024.  The two matmuls carry row groups
                # (0,0) and (64,0) and execute concurrently on the PE array.
                ps = psS.tile([128, LQ], f32, tag="psS", name="psSp")
                for po in range(2):
                    nc.tensor.matmul(
                        ps[:, po * 512:(po + 1) * 512],
                        lhsT=kT_sb[64 * po:64 * po + 64, ct2,
                                   lkt * 128:(lkt + 1) * 128],
                        rhs=qT_sb[64 * po:64 * po + 64, ct2,
                                  ci * LQ + sl * 512:ci * LQ + (sl + 1) * 512],
                        start=True, stop=True,
                    )
                return ps

            def normalize(ps_t, ci, ct2, po, sl, act_sums=False):
                sums = smpool.tile([1, 512], f32, tag="sums")
                if act_sums:
                    nc.scalar.copy(sums, ps_t[64:65, 0:512])
                else:
                    nc.vector.tensor_copy(sums, ps_t[64:65, 0:512])
                recip = smpool.tile([1, 512], f32, tag="recip")
                nc.vector.reciprocal_approx_fast(recip, sums)
                bcast = smpool.tile([64, 512], f32, tag="bcast")
                nc.gpsimd.partition_broadcast(bcast, recip)
                nc.vector.tensor_mul(
                    oT[ci][64 * po:64 * po + 64, ct2,
                           sl * 512:(sl + 1) * 512], ps_t[0:64, 0:512], bcast)



            def s3_piece(ci, mt, act_copy=False, ps_tags=None):
                # out rows [ci*LQ + mt*128 : +128], full H width.  In-stream
                # pieces borrow the opposite sl-phase's two idle psO banks
                # (ps_tags) so the psS ring's S-pair cadence is untouched;
                # tail pieces use the then-idle psS ring.
                osb = opool.tile([128, LQ], bf16, tag="osb")
                if ps_tags is None:
                    pso = psS.tile([128, LQ], f32, tag="psS",
                                   name=f"s3ps{ci}_{mt}")
                    halves = [pso[:, 0:512], pso[:, 512:1024]]
                else:
                    halves = [psO.tile([128, 512], f32, tag=t,
                                       name=f"s3ps{ci}_{mt}_{nt}")
                              for nt, t in enumerate(ps_tags)]
                for nt in range(2):
                    for kt in range(2):
                        nc.tensor.matmul(
                            halves[nt],
                            lhsT=oT[ci][:, kt, mt * 128:(mt + 1) * 128],
                            rhs=wo_sb[:, kt, nt * 512:(nt + 1) * 512],
                            start=(kt == 0), stop=(kt == 1),
                        )
                if ps_tags is None:
                    if act_copy:
                        nc.scalar.copy(osb, pso)
                    else:
                        nc.vector.tensor_copy(osb, pso)
                else:
                    for nt in range(2):
                        nc.vector.tensor_copy(
                            osb[:, nt * 512:(nt + 1) * 512], halves[nt])
                nc.sync.dma_start(
                    out[ci * LQ + mt * 128:ci * LQ + (mt + 1) * 128, :], osb)

            pipe = {}

            # ---- startup: only what the first slots strictly need; the
            # first S pair goes into the PE FIFO straight after the Q
            # projection it reads so exp(0) isn't queued behind V/Q-sl1 ----
            proj_group(wk_sb, yT_sb, kT_sb, 0, 0, 0)   # K ct0 lk 0:512
            proj_group(wq_sb, xT_sb, qT_sb, 0, 0, 0)   # Q ct0 lq 0:512
            pipe["ps"] = emit_S_pair(0, 0, 0, 0)
            proj_group(wq_sb, xT_sb, qT_sb, 0, 0, 1)   # Q ct0 lq 512:1024
            # V(0) first half here; its second half is seg1's slot-0 hook
            # so it lands in the PE FIFO before the O matmul that reads it
            v0a, v0b = v2(0)
            v0a()

            def seg(ci, ct2, sched, nxt, sl_major=False):
                # slot k -> (lkt, sl).  lkt-major relaxes the K-projection
                # and V deadlines (default); sl-major closes the sl0 O
                # accumulators mid-segment so the chunk's output projection
                # can start inside the stream (used for the last segment).
                if sl_major:
                    order = [(k % NKT, k // NKT) for k in range(32)]
                else:
                    order = [(k // 2, k % 2) for k in range(32)]
                ps_t = {}

                def get_ps(sl, po):
                    # claim the accumulator at first use: in sl-major order
                    # the sl1 tiles are claimed only at slot 16, AFTER any
                    # hook that borrowed those banks during the sl0 phase
                    if (sl, po) not in ps_t:
                        ps_t[(sl, po)] = psO.tile(
                            [128, 512], f32, tag=f"psO{sl}{po}",
                            name=f"psO{ci}{ct2}{sl}{po}")
                    return ps_t[(sl, po)]

                if not sl_major:
                    for sl in range(2):
                        for po in range(2):
                            get_ps(sl, po)
                for k in range(32):
                    lkt, sl = order[k]
                    ps = pipe.pop("ps")
                    pt = ptpool.tile([128, LQ], bf16, tag="pt")
                    nc.scalar.activation(pt, ps, Exp)
                    if k + 1 < 32:
                        nl, nsl = order[k + 1]
                        pipe["ps"] = emit_S_pair(ci, ct2, nsl, nl)
                    elif nxt is not None:
                        pipe["ps"] = emit_S_pair(nxt[0], nxt[1], 0, 0)
                    if k == 2 and "defer" in pipe:
                        pipe.pop("defer")()
                    for job in sched.get(k, ()):
                        job()
                    for po in range(2):
                        h = 2 * ct2 + po
                        nc.tensor.matmul(
                            get_ps(sl, po)[0:65, 0:512],
                            lhsT=vaug_sb[:, lkt, h * 65:(h + 1) * 65],
                            rhs=pt[:, po * 512:(po + 1) * 512],
                            start=(lkt == 0), stop=(lkt == NKT - 1),
                        )
                    if lkt == NKT - 1:
                        if sl == 1 and nxt is None:
                            # final segment: sums copies on the then-idle
                            # ScalarE (fusing the two chains into one wide
                            # gpsimd broadcast was tried and SLOWED the
                            # whole stream ~20% -- the extra SBUF tags shift
                            # tile placement into an ACT-hostile region)
                            for po in range(2):
                                normalize(ps_t[(1, po)], ci, ct2, po, 1,
                                          act_sums=True)
                            continue
                        # the very last chain of a segment is deferred into
                        # the next segment's slot 2: it shortens the DVE
                        # queue at the boundary, where the next segment's
                        # first borrowed-psum hooks wait on chain completion
                        for po in range(2):
                            if sl == 1 and po == 1 and nxt is not None:
                                t = ps_t[(sl, po)]
                                pipe["defer"] = (
                                    lambda t=t, a=ci, b=ct2, c=po, d=sl:
                                    normalize(t, a, b, c, d))
                            else:
                                normalize(ps_t[(sl, po)], ci, ct2, po, sl)

            def sched_pairs(pairs):
                # pairs: list of ((thunk_a, thunk_b), (slot_a, slot_b))
                sched = {}
                for (a, b), (sa, sb_) in pairs:
                    sched.setdefault(sa, []).append(a)
                    sched.setdefault(sb_, []).append(b)
                return sched

            # SEG1 (pair0, chunk0), lkt-major: all of V + remaining K ct0 +
            # Q ct0 lh1.  V halves 1/slot; K/Q halves overlay (those slots
            # run PE-paced).  Hook psum comes from the psS ring here (no
            # idle psO banks in lkt-major order).
            seg1 = sched_pairs(
                [((v0b, lambda: None), (0, 1))] +
                [(v2(j), (2 * j - 2, 2 * j - 1)) for j in range(1, 16)] +
                [(pj2(wk_sb, yT_sb, kT_sb, 0, 0, 1), (3, 5)),
                 (pj2(wk_sb, yT_sb, kT_sb, 0, 1, 0), (9, 11)),
                 (pj2(wk_sb, yT_sb, kT_sb, 0, 1, 1), (15, 17)),
                 (pj2(wq_sb, xT_sb, qT_sb, 0, 1, 0), (24, 26))])
            # SEG2 (pair0, chunk1), sl-major: Q ct0 lh1 sl1 (needed by this
            # segment's own sl1 phase), K ct1, Q ct1 lh0 — all borrowing
            # the opposite phase's idle psO banks for projection psum
            seg2 = sched_pairs(
                [(pj2(wk_sb, yT_sb, kT_sb, 1, 0, 0, "psO10"), (6, 8)),
                 (pj2(wq_sb, xT_sb, qT_sb, 0, 1, 1, "psO11"), (7, 9)),
                 (pj2(wk_sb, yT_sb, kT_sb, 1, 0, 1, "psO10"), (10, 12)),
                 (pj2(wk_sb, yT_sb, kT_sb, 1, 1, 0, "psO11"), (11, 13)),
                 (pj2(wk_sb, yT_sb, kT_sb, 1, 1, 1, "psO00"), (18, 20)),
                 (pj2(wq_sb, xT_sb, qT_sb, 1, 0, 0, "psO01"), (22, 24)),
                 (pj2(wq_sb, xT_sb, qT_sb, 1, 0, 1, "psO00"), (26, 28))])
            # SEG3 (pair1, chunk0), sl-major: Q ct1 lh1, then the sl0 half
            # of chunk-0's output projection (this segment's own sl0
            # normalize completes mid-segment)
            seg3 = sched_pairs(
                [(pj2(wq_sb, xT_sb, qT_sb, 1, 1, 0, "psO10"), (6, 8)),
                 (pj2(wq_sb, xT_sb, qT_sb, 1, 1, 1, "psO11"), (10, 12))])
            for mt, s in zip(range(4), (20, 22, 24, 26)):
                seg3[s] = [(lambda mt=mt: s3_piece(
                    0, mt, ps_tags=("psO00", "psO01")))]
            # SEG4 (pair1, chunk1), sl-major: rest of chunk-0's output
            # projection + the sl0 half of chunk-1's
            seg4 = {}
            for mt, s in zip(range(4, 8), (6, 8, 10, 12)):
                seg4[s] = [(lambda mt=mt: s3_piece(
                    0, mt, ps_tags=("psO10", "psO11")))]
            for mt, s in zip(range(4), (20, 22, 24, 26)):
                seg4[s] = [(lambda mt=mt: s3_piece(
                    1, mt, ps_tags=("psO00", "psO01")))]

            seg(0, 0, seg1, nxt=(1, 0))
            seg(1, 0, seg2, nxt=(0, 1), sl_major=True)
            seg(0, 1, seg3, nxt=(1, 1), sl_major=True)
            seg(1, 1, seg4, nxt=None, sl_major=True)
            # warm bridge: dummy matmuls keep the HAM clock-gate open while
            # the final sl1 normalize chains run on DVE/GpSimd, so the tail
            # output-projection matmuls execute at 2.4 GHz
            wps = psS.tile([128, LQ], f32, tag="psS", name="warmtail")
            for _i in range(10):
                nc.tensor.matmul(
                    wps[:, 0:512],
                    lhsT=scratch[:, 0:128], rhs=scratch[:, 0:512],
                    start=True, stop=True,
                )
            # tail: remaining chunk-1 output projection.  Both ScalarE and
            # DVE are idle once the final chains drain -> alternate the
            # copies so they pipeline two-wide behind the matmuls.
            for mt in range(4, LQ // 128):
                s3_piece(1, mt, act_copy=bool(mt % 2 == 0))
    nc.compile()
    return nc


def _get_nc():
    if "nc" not in _CACHE:
        _CACHE["nc"] = _build()
    return _CACHE["nc"]


def _pack_pm(a, t):
    # [t*128, N] -> [128, t, N] partition-major
    return a.reshape(t, 128, -1).transpose(1, 0, 2)


def _pack_act(a):
    # x[b] [L, H] -> xT packed [128, NLQ(lh), 2(sl), 8(t), 512] bf16
    v = _pack_pm(np.ascontiguousarray(a.T), 8)          # [128, 8, L]
    v = v.reshape(128, 8, NLQ, 2, 512).transpose(0, 2, 3, 1, 4)
    return np.ascontiguousarray(v).astype(BF16)


def _in_maps(x, y, Wq, Wk, Wv, Wo):
    maps = []
    for core in range(8):
        b, g = core // GP, core % GP
        cs = slice(g * CH, (g + 1) * CH)
        maps.append({
            "xT": _pack_act(x[b]),
            "yT": _pack_act(y[b]),
            "wq": np.ascontiguousarray(
                _pack_pm(Wq[:, cs] * np.float32(0.125), 8)).astype(BF16),
            "wk": np.ascontiguousarray(_pack_pm(Wk[:, cs], 8)).astype(BF16),
            "wv": np.ascontiguousarray(_pack_pm(Wv[:, cs], 8)).astype(BF16),
            "wo": np.ascontiguousarray(_pack_pm(Wo[cs, :], 2)).astype(BF16),
        })
    return maps


def _install_ntff_hook():
    """Provide the antenv.axon_hooks shim missing from this container so
    run_bass_kernel_spmd(trace=True) can drive NTFF profiling via ctypes."""
    import sys
    import types
    try:
        from antenv.axon_hooks import get_axon_ntff_profile_hook  # noqa: F401
        return
    except ImportError:
        pass
    from trn_agent_boot.trn_boot import _ntff_profile_via_ctypes
    hook = _ntff_profile_via_ctypes("/opt/axon/libaxon_pjrt.so")
    mod = types.ModuleType("antenv.axon_hooks")
    mod.get_axon_ntff_profile_hook = lambda: hook
    mod.set_axon_ntff_profile_hook = lambda h: None
    sys.modules["antenv.axon_hooks"] = mod


def _run(inputs, trace=False):
    from concourse import bass_utils

    if trace:
        _install_ntff_hook()

    x, y, bias = inputs["x"], inputs["y"], inputs["bias"]
    if np.count_nonzero(np.asarray(bias)):
        raise NotImplementedError("nonzero attention bias not supported")
    nc = _get_nc()
    maps = _in_maps(np.asarray(x, np.float32), np.asarray(y, np.float32),
                    np.asarray(inputs["Wq"], np.float32),
                    np.asarray(inputs["Wk"], np.float32),
                    np.asarray(inputs["Wv"], np.float32),
                    np.asarray(inputs["Wo"], np.float32))
    res = bass_utils.run_bass_kernel_spmd(
        nc, maps, list(range(8)), trace=trace)
    out = np.zeros((B, L, H), np.float32)
    for core in range(8):
        out[core // GP] += np.asarray(res.results[core]["out"], np.float32)
    return out, res


def kernel(**inputs):
    out, _ = _run(inputs, trace=False)
    return out


# revision 53
# speedup vs baseline: 1.1854x; 1.0023x over previous
"""Multi-head attention (B=2, L=2048, H=1024, NH=16) on 8 TRN2 NeuronCores.

Sharding: data-parallel over batch (2) x tensor-parallel over heads (4 groups
of 4 heads).  core = b*4 + g handles batch b, heads [4g, 4g+4).  Wq/Wk/Wv are
split column-wise, Wo row-wise; each core produces a partial [L, H] output
that the host sums per batch (the row-parallel all-reduce done host-side).

Device math (per core), all matmuls bf16 inputs / fp32 PSUM accumulation:
  QT = (Wq*0.125)^T x^T          [256, 2048]  (softmax scale folded into Wq)
  KT = Wk^T y^T                  [256, 2048]
  V  = y Wv                      [2048, 256] stored as V_aug [lk, 4*(64+1)]
                                 with a ones column per head
  attention runs in HEAD-PAIR slots: heads (2j, 2j+1) live at partitions
  0:64 / 64:128 of QT/KT, so their S^T matmuls (contraction d=64) carry
  tile_position row groups (0,0)/(64,0) and execute CONCURRENTLY on the two
  row-halves of the PE array (~386ns for the pair vs 2x379 serial).
  Each slot (lkt, sl):
    S^T_a -> psS[:, 0:512], S^T_b -> psS[:, 512:1024]   (one paired burst)
    pt = exp(psS[:, 0:1024])     one FD=1024 ACTIVATE for both heads
    O^T_h[65, 512] += V_aug_h^T pt[:, po*512:]          (row 64 = softmax sums)
  128 slots in 4 segments (pair0-c0 lkt-major, then pair0-c1 / pair1-c0 /
  pair1-c1 sl-major).  PSUM: psS double-buffered [128,1024] (4 banks) +
  four single-bank [128,512] O accumulators keyed by (sl, head-parity).
  sl-major segments close their sl0 accumulators mid-segment, which (a)
  staggers the psO handover so segment boundaries don't stall, and (b)
  frees two banks that in-stream hook work (projections, output-projection
  pieces) borrows for its psum -- keeping the psS ring free so the S-pair /
  exp cadence never hiccups.  The last normalize chain of each segment is
  deferred into the next segment's slot 2 to keep the boundary DVE queue
  short.  V and the Q/K ct1 projections ride as hook thunks scheduled into
  specific slots (V's second half always lands in the PE FIFO before the
  O matmul that consumes it).  Dummy matmuls on scratch bridge the
  DMA-bound startup and the tail normalize window so the PE HAM clock-gate
  stays at 2.4 GHz.  Output partials are staged and DMA'd as bf16; the
  host accumulates the 4 head-group partials per batch in fp32.
"""

import numpy as np
import ml_dtypes

B, L, H, NH, D = 2, 2048, 1024, 16, 64
GP = 4            # head-groups (tensor-parallel factor)
CH = H // GP      # 256 local projection cols per core
HL = NH // GP     # 4 local heads
LQ = 1024         # lq chunk size
NLQ = L // LQ
NKT = L // 128    # 16 lk tiles
BF16 = ml_dtypes.bfloat16

_CACHE = {}


def _build():
    import concourse.mybir as mybir
    import concourse.tile as tile
    from concourse import bacc

    dt = mybir.dt
    f32, bf16 = dt.float32, dt.bfloat16
    Exp = mybir.ActivationFunctionType.Exp

    nc = bacc.Bacc("TRN2", target_bir_lowering=False, debug=False)
    # all inputs host-packed partition-major so each DMA is 128 long
    # contiguous runs (SP descriptor generation is the startup bottleneck)
    xT = nc.declare_dram_parameter("xT", [128, NLQ, 2, 8, 512], bf16,
                                   isOutput=False)
    yT = nc.declare_dram_parameter("yT", [128, NLQ, 2, 8, 512], bf16,
                                   isOutput=False)
    wq = nc.declare_dram_parameter("wq", [128, 8, CH], bf16, isOutput=False)
    wk = nc.declare_dram_parameter("wk", [128, 8, CH], bf16, isOutput=False)
    wv = nc.declare_dram_parameter("wv", [128, 8, CH], bf16, isOutput=False)
    wo = nc.declare_dram_parameter("wo", [128, 2, H], bf16, isOutput=False)
    # bf16 partial output: host accumulates the 4 head-group partials in
    # fp32, so the bf16 rounding (~0.4% per partial) is well inside budget
    out = nc.declare_dram_parameter("out", [L, H], bf16, isOutput=True)

    with tile.TileContext(nc) as tc:
        with (
            tc.tile_pool(name="w", bufs=1) as wpool,
            tc.tile_pool(name="acts", bufs=1) as apool,
            tc.tile_pool(name="psS", bufs=2, space="PSUM") as psS,
            tc.tile_pool(name="psO", bufs=1, space="PSUM") as psO,
            tc.tile_pool(name="pt", bufs=6) as ptpool,
            tc.tile_pool(name="oT", bufs=2) as otpool,
            tc.tile_pool(name="sm", bufs=3) as smpool,
            tc.tile_pool(name="osb", bufs=4) as opool,
        ):
            # prefetch the exp activation table while input DMAs run
            dummy = smpool.tile([1, 8], f32, tag="dummy")
            nc.vector.memset(dummy, 0.0)
            nc.scalar.activation(dummy, dummy, Exp)

            # PE warm-up fodder: matmuls on a memset scratch keep the HAM
            # activity window busy through the DMA-bound startup so the real
            # stream starts (and stays) at 2.4 GHz.
            scratch = apool.tile([128, 512], bf16, tag="scratch")
            nc.vector.memset(scratch, 0.0)

            # ---- input DMAs ordered by first use: weights, then the
            # activation chunks the startup projections + first slots need -
            # DMA order follows the exp(0) dependency chain exactly:
            # wk -> y00 (K1 runs while wq/x00 land) -> wq -> x00 (Q1) ->
            # then everything else.  y00/x00 land in two half-MB pieces so
            # the projections start on the first half (per-subtile deps)
            # without paying per-ht Sync dispatch cost (~0.7us/dma_start).
            wk_sb = wpool.tile([128, 8, CH], bf16, tag="wk")
            nc.sync.dma_start(wk_sb, wk[:, :, :])
            yT_sb = apool.tile([128, NLQ, 2, 8, 512], bf16, tag="yT")
            xT_sb = apool.tile([128, NLQ, 2, 8, 512], bf16, tag="xT")
            nc.sync.dma_start(yT_sb[:, 0, 0, 0:4], yT[:, 0, 0, 0:4])
            nc.sync.dma_start(yT_sb[:, 0, 0, 4:8], yT[:, 0, 0, 4:8])
            wq_sb = wpool.tile([128, 8, CH], bf16, tag="wq")
            nc.sync.dma_start(wq_sb, wq[:, :, :])
            nc.sync.dma_start(xT_sb[:, 0, 0, 0:4], xT[:, 0, 0, 0:4])
            nc.sync.dma_start(xT_sb[:, 0, 0, 4:8], xT[:, 0, 0, 4:8])
            wv_sb = wpool.tile([128, 8, CH], bf16, tag="wv")
            nc.sync.dma_start(wv_sb, wv[:, :, :])
            nc.sync.dma_start(xT_sb[:, 0, 1], xT[:, 0, 1])
            nc.sync.dma_start(yT_sb[:, 0, 1], yT[:, 0, 1])
            nc.sync.dma_start(yT_sb[:, 1, 0], yT[:, 1, 0])
            nc.sync.dma_start(yT_sb[:, 1, 1], yT[:, 1, 1])
            nc.sync.dma_start(xT_sb[:, 1, 0], xT[:, 1, 0])
            nc.sync.dma_start(xT_sb[:, 1, 1], xT[:, 1, 1])
            # (y00/x00/x01 first: the startup K/Q projections gate exp(0))
            wo_sb = wpool.tile([128, 2, H], bf16, tag="wo")
            nc.sync.dma_start(wo_sb, wo[:, :, :])

            # warm-up matmuls (no data deps beyond the scratch memset):
            # enough to flip the HAM clock-gate, short enough to drain
            # before y00's first half lands even on a fast DMA run
            for _w in range(2):
                wps = psS.tile([128, LQ], f32, tag="psS", name=f"warm{_w}")
                for _i in range(4):
                    nc.tensor.matmul(
                        wps[:, 0:512],
                        lhsT=scratch[:, 0:128], rhs=scratch[:, 0:512],
                        start=True, stop=True,
                    )

            qT_sb = apool.tile([128, 2, L], bf16, tag="qT")
            kT_sb = apool.tile([128, 2, L], bf16, tag="kT")
            vaug_sb = apool.tile([128, NKT, HL * 65], bf16, tag="vaug")

            def proj_group(w_sb, act_sb, dst, ct, lh, sl):
                # dst[:, ct, lh*LQ+sl*512 : +512] via one 8-matmul psum group
                ps = psS.tile([128, LQ], f32, tag="psS", name="projps")
                off = lh * LQ + sl * 512
                for ht in range(8):
                    nc.tensor.matmul(
                        ps[:, 0:512],
                        lhsT=w_sb[:, ht, ct * 128:(ct + 1) * 128],
                        rhs=act_sb[:, lh, sl, ht, :],
                        start=(ht == 0), stop=(ht == 7),
                    )
                nc.vector.tensor_copy(dst[:, ct, off:off + 512], ps[:, 0:512])

            def pj2(w_sb, act_sb, dst, ct, lh, sl, ps_tag=None):
                # one projection psum group split into two 4-matmul thunks.
                # ps_tag borrows an idle psO bank (sl-major segments always
                # have the opposite phase's banks free) instead of inserting
                # into the psS ring, which would stall the S-pair cadence.
                cell = {}

                def half(r):
                    def thunk():
                        if r == 0:
                            if ps_tag is None:
                                cell["ps"] = psS.tile(
                                    [128, LQ], f32, tag="psS",
                                    name=f"pjps{ct}_{lh}_{sl}_{id(w_sb) % 97}")
                            else:
                                cell["ps"] = psO.tile(
                                    [128, 512], f32, tag=ps_tag,
                                    name=f"pjps{ct}_{lh}_{sl}_{id(w_sb) % 97}")
                        ps = cell["ps"]
                        for ht in range(4 * r, 4 * r + 4):
                            nc.tensor.matmul(
                                ps[:, 0:512],
                                lhsT=w_sb[:, ht, ct * 128:(ct + 1) * 128],
                                rhs=act_sb[:, lh, sl, ht, :],
                                start=(ht == 0), stop=(ht == 7),
                            )
                        if r == 1:
                            nc.vector.tensor_copy(
                                dst[:, ct, lh * LQ + sl * 512:
                                    lh * LQ + (sl + 1) * 512], ps[:, 0:512])
                    return thunk
                return half(0), half(1)

            def v2(lkt):
                # one lk tile of V_aug[lk, 4*(64+1)] bf16, split in 2 thunks
                cell = {}

                def half(r):
                    def thunk():
                        if r == 0:
                            cell["ps"] = psS.tile(
                                [128, LQ], f32, tag="psS", name=f"vps{lkt}")
                        psv = cell["ps"]
                        for ht in range(4 * r, 4 * r + 4):
                            nc.tensor.matmul(
                                psv[:, :CH],
                                lhsT=yT_sb[:, lkt // 8, (lkt % 8) // 4, ht,
                                           (lkt % 4) * 128:(lkt % 4 + 1) * 128],
                                rhs=wv_sb[:, ht, :],
                                start=(ht == 0), stop=(ht == 7),
                            )
                        if r == 1:
                            vh = vaug_sb[:, lkt, :].rearrange(
                                "p (h e) -> p h e", h=HL)
                            nc.vector.tensor_copy(
                                vh[:, :, 0:64],
                                psv[:, :CH].rearrange("p (h e) -> p h e", h=HL))
                            nc.vector.memset(vh[:, :, 64], 1.0)
                    return thunk
                return half(0), half(1)

            def v_full(lkt):
                a, b = v2(lkt)
                a()
                b()

            oT = [otpool.tile([128, 2, LQ], bf16, tag="oT", name=f"oT{i}")
                  for i in range(NLQ)]

            def emit_S_pair(ci, ct2, sl, lkt):
                # both heads of the pair in one psS tile: po=0 -> cols 0:512,
                # po=1 -> cols 512:1024.  The two matmuls carry row groups
                # (0,0) and (64,0) and execute concurrently on the PE array.
                ps = psS.tile([128, LQ], f32, tag="psS", name="psSp")
                for po in range(2):
                    nc.tensor.matmul(
                        ps[:, po * 512:(po + 1) * 512],
                        lhsT=kT_sb[64 * po:64 * po + 64, ct2,
                                   lkt * 128:(lkt + 1) * 128],
                        rhs=qT_sb[64 * po:64 * po + 64, ct2,
                                  ci * LQ + sl * 512:ci * LQ + (sl + 1) * 512],
                        start=True, stop=True,
                    )
                return ps

            def normalize(ps_t, ci, ct2, po, sl, act_sums=False):
                sums = smpool.tile([1, 512], f32, tag="sums")
                if act_sums:
                    nc.scalar.copy(sums, ps_t[64:65, 0:512])
                else:
                    nc.vector.tensor_copy(sums, ps_t[64:65, 0:512])
                recip = smpool.tile([1, 512], f32, tag="recip")
                nc.vector.reciprocal_approx_fast(recip, sums)
                bcast = smpool.tile([64, 512], f32, tag="bcast")
                nc.gpsimd.partition_broadcast(bcast, recip)
                nc.vector.tensor_mul(
                    oT[ci][64 * po:64 * po + 64, ct2,
                           sl * 512:(sl + 1) * 512], ps_t[0:64, 0:512], bcast)



            def s3_piece(ci, mt, act_copy=False, ps_tags=None):
                # out rows [ci*LQ + mt*128 : +128], full H width.  In-stream
                # pieces borrow the opposite sl-phase's two idle psO banks
                # (ps_tags) so the psS ring's S-pair cadence is untouched;
                # tail pieces use the then-idle psS ring.
                osb = opool.tile([128, LQ], bf16, tag="osb")
                if ps_tags is None:
                    pso = psS.tile([128, LQ], f32, tag="psS",
                                   name=f"s3ps{ci}_{mt}")
                    halves = [pso[:, 0:512], pso[:, 512:1024]]
                else:
                    halves = [psO.tile([128, 512], f32, tag=t,
                                       name=f"s3ps{ci}_{mt}_{nt}")
                              for nt, t in enumerate(ps_tags)]
                for nt in range(2):
                    for kt in range(2):
                        nc.tensor.matmul(
                            halves[nt],
                            lhsT=oT[ci][:, kt, mt * 128:(mt + 1) * 128],
                            rhs=wo_sb[:, kt, nt * 512:(nt + 1) * 512],
                            start=(kt == 0), stop=(kt == 1),
                        )
                if ps_tags is None:
                    if act_copy == "split":
                        # last piece: halves on both idle engines in parallel
                        nc.vector.tensor_copy(osb[:, 0:512], pso[:, 0:512])
                        nc.scalar.copy(osb[:, 512:1024], pso[:, 512:1024])
                    elif act_copy:
                        nc.scalar.copy(osb, pso)
                    else:
                        nc.vector.tensor_copy(osb, pso)
                else:
                    for nt in range(2):
                        nc.vector.tensor_copy(
                            osb[:, nt * 512:(nt + 1) * 512], halves[nt])
                nc.sync.dma_start(
                    out[ci * LQ + mt * 128:ci * LQ + (mt + 1) * 128, :], osb)

            pipe = {}

            # ---- startup: only what the first slots strictly need; the
            # first S pair goes into the PE FIFO straight after the Q
            # projection it reads so exp(0) isn't queued behind V/Q-sl1 ----
            proj_group(wk_sb, yT_sb, kT_sb, 0, 0, 0)   # K ct0 lk 0:512
            proj_group(wq_sb, xT_sb, qT_sb, 0, 0, 0)   # Q ct0 lq 0:512
            pipe["ps"] = emit_S_pair(0, 0, 0, 0)
            proj_group(wq_sb, xT_sb, qT_sb, 0, 0, 1)   # Q ct0 lq 512:1024
            # V(0) first half here; its second half is seg1's slot-0 hook
            # so it lands in the PE FIFO before the O matmul that reads it
            v0a, v0b = v2(0)
            v0a()

            def seg(ci, ct2, sched, nxt, sl_major=False):
                # slot k -> (lkt, sl).  lkt-major relaxes the K-projection
                # and V deadlines (default); sl-major closes the sl0 O
                # accumulators mid-segment so the chunk's output projection
                # can start inside the stream (used for the last segment).
                if sl_major:
                    order = [(k % NKT, k // NKT) for k in range(32)]
                else:
                    order = [(k // 2, k % 2) for k in range(32)]
                ps_t = {}

                def get_ps(sl, po):
                    # claim the accumulator at first use: in sl-major order
                    # the sl1 tiles are claimed only at slot 16, AFTER any
                    # hook that borrowed those banks during the sl0 phase
                    if (sl, po) not in ps_t:
                        ps_t[(sl, po)] = psO.tile(
                            [128, 512], f32, tag=f"psO{sl}{po}",
                            name=f"psO{ci}{ct2}{sl}{po}")
                    return ps_t[(sl, po)]

                if not sl_major:
                    for sl in range(2):
                        for po in range(2):
                            get_ps(sl, po)
                for k in range(32):
                    lkt, sl = order[k]
                    ps = pipe.pop("ps")
                    pt = ptpool.tile([128, LQ], bf16, tag="pt")
                    nc.scalar.activation(pt, ps, Exp)
                    if k + 1 < 32:
                        nl, nsl = order[k + 1]
                        pipe["ps"] = emit_S_pair(ci, ct2, nsl, nl)
                    elif nxt is not None:
                        pipe["ps"] = emit_S_pair(nxt[0], nxt[1], 0, 0)
                    if k == 2 and "defer" in pipe:
                        pipe.pop("defer")()
                    for job in sched.get(k, ()):
                        job()
                    for po in range(2):
                        h = 2 * ct2 + po
                        nc.tensor.matmul(
                            get_ps(sl, po)[0:65, 0:512],
                            lhsT=vaug_sb[:, lkt, h * 65:(h + 1) * 65],
                            rhs=pt[:, po * 512:(po + 1) * 512],
                            start=(lkt == 0), stop=(lkt == NKT - 1),
                        )
                    if lkt == NKT - 1:
                        if sl == 1 and nxt is None:
                            # final segment: sums copies on the then-idle
                            # ScalarE (fusing the two chains into one wide
                            # gpsimd broadcast was tried and SLOWED the
                            # whole stream ~20% -- the extra SBUF tags shift
                            # tile placement into an ACT-hostile region)
                            for po in range(2):
                                normalize(ps_t[(1, po)], ci, ct2, po, 1,
                                          act_sums=True)
                            continue
                        # the very last chain of a segment is deferred into
                        # the next segment's slot 2: it shortens the DVE
                        # queue at the boundary, where the next segment's
                        # first borrowed-psum hooks wait on chain completion
                        for po in range(2):
                            if sl == 1 and po == 1 and nxt is not None:
                                t = ps_t[(sl, po)]
                                pipe["defer"] = (
                                    lambda t=t, a=ci, b=ct2, c=po, d=sl:
                                    normalize(t, a, b, c, d))
                            else:
                                normalize(ps_t[(sl, po)], ci, ct2, po, sl)

            def sched_pairs(pairs):
                # pairs: list of ((thunk_a, thunk_b), (slot_a, slot_b))
                sched = {}
                for (a, b), (sa, sb_) in pairs:
                    sched.setdefault(sa, []).append(a)
                    sched.setdefault(sb_, []).append(b)
                return sched

            # SEG1 (pair0, chunk0), lkt-major: all of V + remaining K ct0 +
            # Q ct0 lh1.  V halves 1/slot; K/Q halves overlay (those slots
            # run PE-paced).  Hook psum comes from the psS ring here (no
            # idle psO banks in lkt-major order).
            seg1 = sched_pairs(
                [((v0b, lambda: None), (0, 1))] +
                [(v2(j), (2 * j - 2, 2 * j - 1)) for j in range(1, 16)] +
                [(pj2(wk_sb, yT_sb, kT_sb, 0, 0, 1), (3, 5)),
                 (pj2(wk_sb, yT_sb, kT_sb, 0, 1, 0), (9, 11)),
                 (pj2(wk_sb, yT_sb, kT_sb, 0, 1, 1), (15, 17)),
                 (pj2(wq_sb, xT_sb, qT_sb, 0, 1, 0), (24, 26))])
            # SEG2 (pair0, chunk1), sl-major: Q ct0 lh1 sl1 (needed by this
            # segment's own sl1 phase), K ct1, Q ct1 lh0 — all borrowing
            # the opposite phase's idle psO banks for projection psum
            seg2 = sched_pairs(
                [(pj2(wk_sb, yT_sb, kT_sb, 1, 0, 0, "psO10"), (6, 8)),
                 (pj2(wq_sb, xT_sb, qT_sb, 0, 1, 1, "psO11"), (7, 9)),
                 (pj2(wk_sb, yT_sb, kT_sb, 1, 0, 1, "psO10"), (10, 12)),
                 (pj2(wk_sb, yT_sb, kT_sb, 1, 1, 0, "psO11"), (11, 13)),
                 (pj2(wk_sb, yT_sb, kT_sb, 1, 1, 1, "psO00"), (18, 20)),
                 (pj2(wq_sb, xT_sb, qT_sb, 1, 0, 0, "psO01"), (22, 24)),
                 (pj2(wq_sb, xT_sb, qT_sb, 1, 0, 1, "psO00"), (26, 28))])
            # SEG3 (pair1, chunk0), sl-major: Q ct1 lh1, then the sl0 half
            # of chunk-0's output projection (this segment's own sl0
            # normalize completes mid-segment)
            seg3 = sched_pairs(
                [(pj2(wq_sb, xT_sb, qT_sb, 1, 1, 0, "psO10"), (6, 8)),
                 (pj2(wq_sb, xT_sb, qT_sb, 1, 1, 1, "psO11"), (10, 12))])
            for mt, s in zip(range(4), (20, 22, 24, 26)):
                seg3[s] = [(lambda mt=mt: s3_piece(
                    0, mt, ps_tags=("psO00", "psO01")))]
            # SEG4 (pair1, chunk1), sl-major: rest of chunk-0's output
            # projection + the sl0 half of chunk-1's
            seg4 = {}
            for mt, s in zip(range(4, 8), (6, 8, 10, 12)):
                seg4[s] = [(lambda mt=mt: s3_piece(
                    0, mt, ps_tags=("psO10", "psO11")))]
            for mt, s in zip(range(4), (20, 22, 24, 26)):
                seg4[s] = [(lambda mt=mt: s3_piece(
                    1, mt, ps_tags=("psO00", "psO01")))]

            seg(0, 0, seg1, nxt=(1, 0))
            seg(1, 0, seg2, nxt=(0, 1), sl_major=True)
            seg(0, 1, seg3, nxt=(1, 1), sl_major=True)
            seg(1, 1, seg4, nxt=None, sl_major=True)
            # warm bridge: dummy matmuls keep the HAM clock-gate open while
            # the final sl1 normalize chains run on DVE/GpSimd, so the tail
            # output-projection matmuls execute at 2.4 GHz
            wps = psS.tile([128, LQ], f32, tag="psS", name="warmtail")
            for _i in range(10):
                nc.tensor.matmul(
                    wps[:, 0:512],
                    lhsT=scratch[:, 0:128], rhs=scratch[:, 0:512],
                    start=True, stop=True,
                )
            # tail: remaining chunk-1 output projection.  Both ScalarE and
            # DVE are idle once the final chains drain -> alternate the
            # copies so they pipeline two-wide behind the matmuls.
            for mt in range(4, LQ // 128):
                s3_piece(1, mt,
                         act_copy="split" if mt == 7 else bool(mt % 2 == 0))
    nc.compile()
    return nc


def _get_nc():
    if "nc" not in _CACHE:
        _CACHE["nc"] = _build()
    return _CACHE["nc"]


def _pack_pm(a, t):
    # [t*128, N] -> [128, t, N] partition-major
    return a.reshape(t, 128, -1).transpose(1, 0, 2)


def _pack_act(a):
    # x[b] [L, H] -> xT packed [128, NLQ(lh), 2(sl), 8(t), 512] bf16
    v = _pack_pm(np.ascontiguousarray(a.T), 8)          # [128, 8, L]
    v = v.reshape(128, 8, NLQ, 2, 512).transpose(0, 2, 3, 1, 4)
    return np.ascontiguousarray(v).astype(BF16)


def _in_maps(x, y, Wq, Wk, Wv, Wo):
    maps = []
    for core in range(8):
        b, g = core // GP, core % GP
        cs = slice(g * CH, (g + 1) * CH)
        maps.append({
            "xT": _pack_act(x[b]),
            "yT": _pack_act(y[b]),
            "wq": np.ascontiguousarray(
                _pack_pm(Wq[:, cs] * np.float32(0.125), 8)).astype(BF16),
            "wk": np.ascontiguousarray(_pack_pm(Wk[:, cs], 8)).astype(BF16),
            "wv": np.ascontiguousarray(_pack_pm(Wv[:, cs], 8)).astype(BF16),
            "wo": np.ascontiguousarray(_pack_pm(Wo[cs, :], 2)).astype(BF16),
        })
    return maps


def _install_ntff_hook():
    """Provide the antenv.axon_hooks shim missing from this container so
    run_bass_kernel_spmd(trace=True) can drive NTFF profiling via ctypes."""
    import sys
    import types
    try:
        from antenv.axon_hooks import get_axon_ntff_profile_hook  # noqa: F401
        return
    except ImportError:
        pass
    from trn_agent_boot.trn_boot import _ntff_profile_via_ctypes
    hook = _ntff_profile_via_ctypes("/opt/axon/libaxon_pjrt.so")
    mod = types.ModuleType("antenv.axon_hooks")
    mod.get_axon_ntff_profile_hook = lambda: hook
    mod.set_axon_ntff_profile_hook = lambda h: None
    sys.modules["antenv.axon_hooks"] = mod


def _run(inputs, trace=False):
    from concourse import bass_utils

    if trace:
        _install_ntff_hook()

    x, y, bias = inputs["x"], inputs["y"], inputs["bias"]
    if np.count_nonzero(np.asarray(bias)):
        raise NotImplementedError("nonzero attention bias not supported")
    nc = _get_nc()
    maps = _in_maps(np.asarray(x, np.float32), np.asarray(y, np.float32),
                    np.asarray(inputs["Wq"], np.float32),
                    np.asarray(inputs["Wk"], np.float32),
                    np.asarray(inputs["Wv"], np.float32),
                    np.asarray(inputs["Wo"], np.float32))
    res = bass_utils.run_bass_kernel_spmd(
        nc, maps, list(range(8)), trace=trace)
    out = np.zeros((B, L, H), np.float32)
    for core in range(8):
        out[core // GP] += np.asarray(res.results[core]["out"], np.float32)
    return out, res


def kernel(**inputs):
    out, _ = _run(inputs, trace=False)
    return out


# revision 54
# speedup vs baseline: 1.1949x; 1.0080x over previous
"""Multi-head attention (B=2, L=2048, H=1024, NH=16) on 8 TRN2 NeuronCores.

Sharding: data-parallel over batch (2) x tensor-parallel over heads (4 groups
of 4 heads).  core = b*4 + g handles batch b, heads [4g, 4g+4).  Wq/Wk/Wv are
split column-wise, Wo row-wise; each core produces a partial [L, H] output
that the host sums per batch (the row-parallel all-reduce done host-side).

Device math (per core), all matmuls bf16 inputs / fp32 PSUM accumulation:
  QT = (Wq*0.125)^T x^T          [256, 2048]  (softmax scale folded into Wq)
  KT = Wk^T y^T                  [256, 2048]
  V  = y Wv                      [2048, 256] stored as V_aug [lk, 4*(64+1)]
                                 with a ones column per head
  attention runs in HEAD-PAIR slots: heads (2j, 2j+1) live at partitions
  0:64 / 64:128 of QT/KT, so their S^T matmuls (contraction d=64) carry
  tile_position row groups (0,0)/(64,0) and execute CONCURRENTLY on the two
  row-halves of the PE array (~386ns for the pair vs 2x379 serial).
  Each slot (lkt, sl):
    S^T_a -> psS[:, 0:512], S^T_b -> psS[:, 512:1024]   (one paired burst)
    pt = exp(psS[:, 0:1024])     one FD=1024 ACTIVATE for both heads
    O^T_h[65, 512] += V_aug_h^T pt[:, po*512:]          (row 64 = softmax sums)
  128 slots in 4 segments (pair0-c0 lkt-major, then pair0-c1 / pair1-c0 /
  pair1-c1 sl-major).  PSUM: psS double-buffered [128,1024] (4 banks) +
  four single-bank [128,512] O accumulators keyed by (sl, head-parity).
  sl-major segments close their sl0 accumulators mid-segment, which (a)
  staggers the psO handover so segment boundaries don't stall, and (b)
  frees two banks that in-stream hook work (projections, output-projection
  pieces) borrows for its psum -- keeping the psS ring free so the S-pair /
  exp cadence never hiccups.  The last normalize chain of each segment is
  deferred into the next segment's slot 2 to keep the boundary DVE queue
  short.  V and the Q/K ct1 projections ride as hook thunks scheduled into
  specific slots (V's second half always lands in the PE FIFO before the
  O matmul that consumes it).  Dummy matmuls on scratch bridge the
  DMA-bound startup and the tail normalize window so the PE HAM clock-gate
  stays at 2.4 GHz.  Output partials are staged and DMA'd as bf16; the
  host accumulates the 4 head-group partials per batch in fp32.
"""

import numpy as np
import ml_dtypes

B, L, H, NH, D = 2, 2048, 1024, 16, 64
GP = 4            # head-groups (tensor-parallel factor)
CH = H // GP      # 256 local projection cols per core
HL = NH // GP     # 4 local heads
LQ = 1024         # lq chunk size
NLQ = L // LQ
NKT = L // 128    # 16 lk tiles
BF16 = ml_dtypes.bfloat16

_CACHE = {}


def _build():
    import concourse.mybir as mybir
    import concourse.tile as tile
    from concourse import bacc

    dt = mybir.dt
    f32, bf16 = dt.float32, dt.bfloat16
    Exp = mybir.ActivationFunctionType.Exp

    nc = bacc.Bacc("TRN2", target_bir_lowering=False, debug=False)
    # all inputs host-packed partition-major so each DMA is 128 long
    # contiguous runs (SP descriptor generation is the startup bottleneck)
    xT = nc.declare_dram_parameter("xT", [128, NLQ, 2, 8, 512], bf16,
                                   isOutput=False)
    yT = nc.declare_dram_parameter("yT", [128, NLQ, 2, 8, 512], bf16,
                                   isOutput=False)
    wq = nc.declare_dram_parameter("wq", [128, 8, CH], bf16, isOutput=False)
    wk = nc.declare_dram_parameter("wk", [128, 8, CH], bf16, isOutput=False)
    wv = nc.declare_dram_parameter("wv", [128, 8, CH], bf16, isOutput=False)
    wo = nc.declare_dram_parameter("wo", [128, 2, H], bf16, isOutput=False)
    # bf16 partial output: host accumulates the 4 head-group partials in
    # fp32, so the bf16 rounding (~0.4% per partial) is well inside budget
    out = nc.declare_dram_parameter("out", [L, H], bf16, isOutput=True)

    with tile.TileContext(nc) as tc:
        with (
            tc.tile_pool(name="w", bufs=1) as wpool,
            tc.tile_pool(name="acts", bufs=1) as apool,
            tc.tile_pool(name="psS", bufs=2, space="PSUM") as psS,
            tc.tile_pool(name="psO", bufs=1, space="PSUM") as psO,
            tc.tile_pool(name="pt", bufs=6) as ptpool,
            tc.tile_pool(name="oT", bufs=2) as otpool,
            tc.tile_pool(name="sm", bufs=3) as smpool,
            tc.tile_pool(name="osb", bufs=4) as opool,
        ):
            # prefetch the exp activation table while input DMAs run
            dummy = smpool.tile([1, 8], f32, tag="dummy")
            nc.vector.memset(dummy, 0.0)
            nc.scalar.activation(dummy, dummy, Exp)

            # PE warm-up fodder: matmuls on a memset scratch keep the HAM
            # activity window busy through the DMA-bound startup so the real
            # stream starts (and stays) at 2.4 GHz.
            scratch = apool.tile([128, 512], bf16, tag="scratch")
            nc.vector.memset(scratch, 0.0)

            # ---- input DMAs ordered by first use: weights, then the
            # activation chunks the startup projections + first slots need -
            # DMA order follows the exp(0) dependency chain exactly:
            # wk -> y00 (K1 runs while wq/x00 land) -> wq -> x00 (Q1) ->
            # then everything else.  y00/x00 land in two half-MB pieces so
            # the projections start on the first half (per-subtile deps)
            # without paying per-ht Sync dispatch cost (~0.7us/dma_start).
            wk_sb = wpool.tile([128, 8, CH], bf16, tag="wk")
            nc.sync.dma_start(wk_sb, wk[:, :, :])
            yT_sb = apool.tile([128, NLQ, 2, 8, 512], bf16, tag="yT")
            xT_sb = apool.tile([128, NLQ, 2, 8, 512], bf16, tag="xT")
            nc.sync.dma_start(yT_sb[:, 0, 0, 0:4], yT[:, 0, 0, 0:4])
            nc.sync.dma_start(yT_sb[:, 0, 0, 4:8], yT[:, 0, 0, 4:8])
            wq_sb = wpool.tile([128, 8, CH], bf16, tag="wq")
            nc.sync.dma_start(wq_sb, wq[:, :, :])
            nc.sync.dma_start(xT_sb[:, 0, 0, 0:4], xT[:, 0, 0, 0:4])
            nc.sync.dma_start(xT_sb[:, 0, 0, 4:8], xT[:, 0, 0, 4:8])
            # x01 right after x00: slot 1's exp gates on Q2 <- x01
            nc.sync.dma_start(xT_sb[:, 0, 1], xT[:, 0, 1])
            wv_sb = wpool.tile([128, 8, CH], bf16, tag="wv")
            nc.sync.dma_start(wv_sb, wv[:, :, :])
            nc.sync.dma_start(yT_sb[:, 0, 1], yT[:, 0, 1])
            nc.sync.dma_start(yT_sb[:, 1, 0], yT[:, 1, 0])
            nc.sync.dma_start(yT_sb[:, 1, 1], yT[:, 1, 1])
            nc.sync.dma_start(xT_sb[:, 1, 0], xT[:, 1, 0])
            nc.sync.dma_start(xT_sb[:, 1, 1], xT[:, 1, 1])
            # (y00/x00/x01 first: the startup K/Q projections gate exp(0))
            wo_sb = wpool.tile([128, 2, H], bf16, tag="wo")
            nc.sync.dma_start(wo_sb, wo[:, :, :])

            # warm-up matmuls (no data deps beyond the scratch memset):
            # enough to flip the HAM clock-gate, short enough to drain
            # before y00's first half lands even on a fast DMA run
            for _w in range(2):
                wps = psS.tile([128, LQ], f32, tag="psS", name=f"warm{_w}")
                for _i in range(4):
                    nc.tensor.matmul(
                        wps[:, 0:512],
                        lhsT=scratch[:, 0:128], rhs=scratch[:, 0:512],
                        start=True, stop=True,
                    )

            qT_sb = apool.tile([128, 2, L], bf16, tag="qT")
            kT_sb = apool.tile([128, 2, L], bf16, tag="kT")
            vaug_sb = apool.tile([128, NKT, HL * 65], bf16, tag="vaug")

            def proj_group(w_sb, act_sb, dst, ct, lh, sl):
                # dst[:, ct, lh*LQ+sl*512 : +512] via one 8-matmul psum group
                ps = psS.tile([128, LQ], f32, tag="psS", name="projps")
                off = lh * LQ + sl * 512
                for ht in range(8):
                    nc.tensor.matmul(
                        ps[:, 0:512],
                        lhsT=w_sb[:, ht, ct * 128:(ct + 1) * 128],
                        rhs=act_sb[:, lh, sl, ht, :],
                        start=(ht == 0), stop=(ht == 7),
                    )
                nc.vector.tensor_copy(dst[:, ct, off:off + 512], ps[:, 0:512])

            def pj2(w_sb, act_sb, dst, ct, lh, sl, ps_tag=None):
                # one projection psum group split into two 4-matmul thunks.
                # ps_tag borrows an idle psO bank (sl-major segments always
                # have the opposite phase's banks free) instead of inserting
                # into the psS ring, which would stall the S-pair cadence.
                cell = {}

                def half(r):
                    def thunk():
                        if r == 0:
                            if ps_tag is None:
                                cell["ps"] = psS.tile(
                                    [128, LQ], f32, tag="psS",
                                    name=f"pjps{ct}_{lh}_{sl}_{id(w_sb) % 97}")
                            else:
                                cell["ps"] = psO.tile(
                                    [128, 512], f32, tag=ps_tag,
                                    name=f"pjps{ct}_{lh}_{sl}_{id(w_sb) % 97}")
                        ps = cell["ps"]
                        for ht in range(4 * r, 4 * r + 4):
                            nc.tensor.matmul(
                                ps[:, 0:512],
                                lhsT=w_sb[:, ht, ct * 128:(ct + 1) * 128],
                                rhs=act_sb[:, lh, sl, ht, :],
                                start=(ht == 0), stop=(ht == 7),
                            )
                        if r == 1:
                            nc.vector.tensor_copy(
                                dst[:, ct, lh * LQ + sl * 512:
                                    lh * LQ + (sl + 1) * 512], ps[:, 0:512])
                    return thunk
                return half(0), half(1)

            def v2(lkt):
                # one lk tile of V_aug[lk, 4*(64+1)] bf16, split in 2 thunks
                cell = {}

                def half(r):
                    def thunk():
                        if r == 0:
                            cell["ps"] = psS.tile(
                                [128, LQ], f32, tag="psS", name=f"vps{lkt}")
                        psv = cell["ps"]
                        for ht in range(4 * r, 4 * r + 4):
                            nc.tensor.matmul(
                                psv[:, :CH],
                                lhsT=yT_sb[:, lkt // 8, (lkt % 8) // 4, ht,
                                           (lkt % 4) * 128:(lkt % 4 + 1) * 128],
                                rhs=wv_sb[:, ht, :],
                                start=(ht == 0), stop=(ht == 7),
                            )
                        if r == 1:
                            vh = vaug_sb[:, lkt, :].rearrange(
                                "p (h e) -> p h e", h=HL)
                            nc.vector.tensor_copy(
                                vh[:, :, 0:64],
                                psv[:, :CH].rearrange("p (h e) -> p h e", h=HL))
                            nc.vector.memset(vh[:, :, 64], 1.0)
                    return thunk
                return half(0), half(1)

            def v_full(lkt):
                a, b = v2(lkt)
                a()
                b()

            oT = [otpool.tile([128, 2, LQ], bf16, tag="oT", name=f"oT{i}")
                  for i in range(NLQ)]

            def emit_S_pair(ci, ct2, sl, lkt):
                # both heads of the pair in one psS tile: po=0 -> cols 0:512,
                # po=1 -> cols 512:1024.  The two matmuls carry row groups
                # (0,0) and (64,0) and execute concurrently on the PE array.
                ps = psS.tile([128, LQ], f32, tag="psS", name="psSp")
                for po in range(2):
                    nc.tensor.matmul(
                        ps[:, po * 512:(po + 1) * 512],
                        lhsT=kT_sb[64 * po:64 * po + 64, ct2,
                                   lkt * 128:(lkt + 1) * 128],
                        rhs=qT_sb[64 * po:64 * po + 64, ct2,
                                  ci * LQ + sl * 512:ci * LQ + (sl + 1) * 512],
                        start=True, stop=True,
                    )
                return ps

            def normalize(ps_t, ci, ct2, po, sl, act_sums=False):
                sums = smpool.tile([1, 512], f32, tag="sums")
                if act_sums:
                    nc.scalar.copy(sums, ps_t[64:65, 0:512])
                else:
                    nc.vector.tensor_copy(sums, ps_t[64:65, 0:512])
                recip = smpool.tile([1, 512], f32, tag="recip")
                nc.vector.reciprocal_approx_fast(recip, sums)
                bcast = smpool.tile([64, 512], f32, tag="bcast")
                nc.gpsimd.partition_broadcast(bcast, recip)
                nc.vector.tensor_mul(
                    oT[ci][64 * po:64 * po + 64, ct2,
                           sl * 512:(sl + 1) * 512], ps_t[0:64, 0:512], bcast)



            def s3_piece(ci, mt, act_copy=False, ps_tags=None):
                # out rows [ci*LQ + mt*128 : +128], full H width.  In-stream
                # pieces borrow the opposite sl-phase's two idle psO banks
                # (ps_tags) so the psS ring's S-pair cadence is untouched;
                # tail pieces use the then-idle psS ring.
                osb = opool.tile([128, LQ], bf16, tag="osb")
                if ps_tags is None:
                    pso = psS.tile([128, LQ], f32, tag="psS",
                                   name=f"s3ps{ci}_{mt}")
                    halves = [pso[:, 0:512], pso[:, 512:1024]]
                else:
                    halves = [psO.tile([128, 512], f32, tag=t,
                                       name=f"s3ps{ci}_{mt}_{nt}")
                              for nt, t in enumerate(ps_tags)]
                for nt in range(2):
                    for kt in range(2):
                        nc.tensor.matmul(
                            halves[nt],
                            lhsT=oT[ci][:, kt, mt * 128:(mt + 1) * 128],
                            rhs=wo_sb[:, kt, nt * 512:(nt + 1) * 512],
                            start=(kt == 0), stop=(kt == 1),
                        )
                if ps_tags is None:
                    if act_copy == "split":
                        # last piece: halves on both idle engines in parallel
                        nc.vector.tensor_copy(osb[:, 0:512], pso[:, 0:512])
                        nc.scalar.copy(osb[:, 512:1024], pso[:, 512:1024])
                    elif act_copy:
                        nc.scalar.copy(osb, pso)
                    else:
                        nc.vector.tensor_copy(osb, pso)
                else:
                    for nt in range(2):
                        nc.vector.tensor_copy(
                            osb[:, nt * 512:(nt + 1) * 512], halves[nt])
                nc.sync.dma_start(
                    out[ci * LQ + mt * 128:ci * LQ + (mt + 1) * 128, :], osb)

            pipe = {}

            # ---- startup: only what the first slots strictly need; the
            # first S pair goes into the PE FIFO straight after the Q
            # projection it reads so exp(0) isn't queued behind V/Q-sl1 ----
            proj_group(wk_sb, yT_sb, kT_sb, 0, 0, 0)   # K ct0 lk 0:512
            proj_group(wq_sb, xT_sb, qT_sb, 0, 0, 0)   # Q ct0 lq 0:512
            pipe["ps"] = emit_S_pair(0, 0, 0, 0)
            proj_group(wq_sb, xT_sb, qT_sb, 0, 0, 1)   # Q ct0 lq 512:1024
            # V(0) first half here; its second half is seg1's slot-0 hook
            # so it lands in the PE FIFO before the O matmul that reads it
            v0a, v0b = v2(0)
            v0a()

            def seg(ci, ct2, sched, nxt, sl_major=False):
                # slot k -> (lkt, sl).  lkt-major relaxes the K-projection
                # and V deadlines (default); sl-major closes the sl0 O
                # accumulators mid-segment so the chunk's output projection
                # can start inside the stream (used for the last segment).
                if sl_major:
                    order = [(k % NKT, k // NKT) for k in range(32)]
                else:
                    order = [(k // 2, k % 2) for k in range(32)]
                ps_t = {}

                def get_ps(sl, po):
                    # claim the accumulator at first use: in sl-major order
                    # the sl1 tiles are claimed only at slot 16, AFTER any
                    # hook that borrowed those banks during the sl0 phase
                    if (sl, po) not in ps_t:
                        ps_t[(sl, po)] = psO.tile(
                            [128, 512], f32, tag=f"psO{sl}{po}",
                            name=f"psO{ci}{ct2}{sl}{po}")
                    return ps_t[(sl, po)]

                if not sl_major:
                    for sl in range(2):
                        for po in range(2):
                            get_ps(sl, po)
                for k in range(32):
                    lkt, sl = order[k]
                    ps = pipe.pop("ps")
                    pt = ptpool.tile([128, LQ], bf16, tag="pt")
                    nc.scalar.activation(pt, ps, Exp)
                    if k + 1 < 32:
                        nl, nsl = order[k + 1]
                        pipe["ps"] = emit_S_pair(ci, ct2, nsl, nl)
                    elif nxt is not None:
                        pipe["ps"] = emit_S_pair(nxt[0], nxt[1], 0, 0)
                    if k == 2 and "defer" in pipe:
                        pipe.pop("defer")()
                    for job in sched.get(k, ()):
                        job()
                    for po in range(2):
                        h = 2 * ct2 + po
                        nc.tensor.matmul(
                            get_ps(sl, po)[0:65, 0:512],
                            lhsT=vaug_sb[:, lkt, h * 65:(h + 1) * 65],
                            rhs=pt[:, po * 512:(po + 1) * 512],
                            start=(lkt == 0), stop=(lkt == NKT - 1),
                        )
                    if lkt == NKT - 1:
                        if sl == 1 and nxt is None:
                            # final segment: sums copies on the then-idle
                            # ScalarE (fusing the two chains into one wide
                            # gpsimd broadcast was tried and SLOWED the
                            # whole stream ~20% -- the extra SBUF tags shift
                            # tile placement into an ACT-hostile region)
                            for po in range(2):
                                normalize(ps_t[(1, po)], ci, ct2, po, 1,
                                          act_sums=True)
                            continue
                        # the very last chain of a segment is deferred into
                        # the next segment's slot 2: it shortens the DVE
                        # queue at the boundary, where the next segment's
                        # first borrowed-psum hooks wait on chain completion
                        for po in range(2):
                            if sl == 1 and po == 1 and nxt is not None:
                                t = ps_t[(sl, po)]
                                pipe["defer"] = (
                                    lambda t=t, a=ci, b=ct2, c=po, d=sl:
                                    normalize(t, a, b, c, d))
                            else:
                                normalize(ps_t[(sl, po)], ci, ct2, po, sl)

            def sched_pairs(pairs):
                # pairs: list of ((thunk_a, thunk_b), (slot_a, slot_b))
                sched = {}
                for (a, b), (sa, sb_) in pairs:
                    sched.setdefault(sa, []).append(a)
                    sched.setdefault(sb_, []).append(b)
                return sched

            # SEG1 (pair0, chunk0), lkt-major: all of V + remaining K ct0 +
            # Q ct0 lh1.  V halves 1/slot; K/Q halves overlay (those slots
            # run PE-paced).  Hook psum comes from the psS ring here (no
            # idle psO banks in lkt-major order).
            seg1 = sched_pairs(
                [((v0b, lambda: None), (0, 1))] +
                [(v2(j), (2 * j - 2, 2 * j - 1)) for j in range(1, 16)] +
                [(pj2(wk_sb, yT_sb, kT_sb, 0, 0, 1), (3, 5)),
                 (pj2(wk_sb, yT_sb, kT_sb, 0, 1, 0), (9, 11)),
                 (pj2(wk_sb, yT_sb, kT_sb, 0, 1, 1), (15, 17)),
                 (pj2(wq_sb, xT_sb, qT_sb, 0, 1, 0), (24, 26))])
            # SEG2 (pair0, chunk1), sl-major: Q ct0 lh1 sl1 (needed by this
            # segment's own sl1 phase), K ct1, Q ct1 lh0 — all borrowing
            # the opposite phase's idle psO banks for projection psum
            seg2 = sched_pairs(
                [(pj2(wk_sb, yT_sb, kT_sb, 1, 0, 0, "psO10"), (6, 8)),
                 (pj2(wq_sb, xT_sb, qT_sb, 0, 1, 1, "psO11"), (7, 9)),
                 (pj2(wk_sb, yT_sb, kT_sb, 1, 0, 1, "psO10"), (10, 12)),
                 (pj2(wk_sb, yT_sb, kT_sb, 1, 1, 0, "psO11"), (11, 13)),
                 (pj2(wk_sb, yT_sb, kT_sb, 1, 1, 1, "psO00"), (18, 20)),
                 (pj2(wq_sb, xT_sb, qT_sb, 1, 0, 0, "psO01"), (22, 24)),
                 (pj2(wq_sb, xT_sb, qT_sb, 1, 0, 1, "psO00"), (26, 28))])
            # SEG3 (pair1, chunk0), sl-major: Q ct1 lh1, then the sl0 half
            # of chunk-0's output projection (this segment's own sl0
            # normalize completes mid-segment)
            seg3 = sched_pairs(
                [(pj2(wq_sb, xT_sb, qT_sb, 1, 1, 0, "psO10"), (6, 8)),
                 (pj2(wq_sb, xT_sb, qT_sb, 1, 1, 1, "psO11"), (10, 12))])
            for mt, s in zip(range(4), (20, 22, 24, 26)):
                seg3[s] = [(lambda mt=mt: s3_piece(
                    0, mt, ps_tags=("psO00", "psO01")))]
            # SEG4 (pair1, chunk1), sl-major: rest of chunk-0's output
            # projection + the sl0 half of chunk-1's
            seg4 = {}
            for mt, s in zip(range(4, 8), (6, 8, 10, 12)):
                seg4[s] = [(lambda mt=mt: s3_piece(
                    0, mt, ps_tags=("psO10", "psO11")))]
            for mt, s in zip(range(4), (20, 22, 24, 26)):
                seg4[s] = [(lambda mt=mt: s3_piece(
                    1, mt, ps_tags=("psO00", "psO01")))]

            seg(0, 0, seg1, nxt=(1, 0))
            seg(1, 0, seg2, nxt=(0, 1), sl_major=True)
            seg(0, 1, seg3, nxt=(1, 1), sl_major=True)
            seg(1, 1, seg4, nxt=None, sl_major=True)
            # warm bridge: dummy matmuls keep the HAM clock-gate open while
            # the final sl1 normalize chains run on DVE/GpSimd, so the tail
            # output-projection matmuls execute at 2.4 GHz
            wps = psS.tile([128, LQ], f32, tag="psS", name="warmtail")
            for _i in range(10):
                nc.tensor.matmul(
                    wps[:, 0:512],
                    lhsT=scratch[:, 0:128], rhs=scratch[:, 0:512],
                    start=True, stop=True,
                )
            # tail: remaining chunk-1 output projection.  Both ScalarE and
            # DVE are idle once the final chains drain -> alternate the
            # copies so they pipeline two-wide behind the matmuls.
            for mt in range(4, LQ // 128):
                s3_piece(1, mt,
                         act_copy="split" if mt == 7 else bool(mt % 2 == 0))
    nc.compile()
    return nc


def _get_nc():
    if "nc" not in _CACHE:
        _CACHE["nc"] = _build()
    return _CACHE["nc"]


def _pack_pm(a, t):
    # [t*128, N] -> [128, t, N] partition-major
    return a.reshape(t, 128, -1).transpose(1, 0, 2)


def _pack_act(a):
    # x[b] [L, H] -> xT packed [128, NLQ(lh), 2(sl), 8(t), 512] bf16
    v = _pack_pm(np.ascontiguousarray(a.T), 8)          # [128, 8, L]
    v = v.reshape(128, 8, NLQ, 2, 512).transpose(0, 2, 3, 1, 4)
    return np.ascontiguousarray(v).astype(BF16)


def _in_maps(x, y, Wq, Wk, Wv, Wo):
    maps = []
    for core in range(8):
        b, g = core // GP, core % GP
        cs = slice(g * CH, (g + 1) * CH)
        maps.append({
            "xT": _pack_act(x[b]),
            "yT": _pack_act(y[b]),
            "wq": np.ascontiguousarray(
                _pack_pm(Wq[:, cs] * np.float32(0.125), 8)).astype(BF16),
            "wk": np.ascontiguousarray(_pack_pm(Wk[:, cs], 8)).astype(BF16),
            "wv": np.ascontiguousarray(_pack_pm(Wv[:, cs], 8)).astype(BF16),
            "wo": np.ascontiguousarray(_pack_pm(Wo[cs, :], 2)).astype(BF16),
        })
    return maps


def _install_ntff_hook():
    """Provide the antenv.axon_hooks shim missing from this container so
    run_bass_kernel_spmd(trace=True) can drive NTFF profiling via ctypes."""
    import sys
    import types
    try:
        from antenv.axon_hooks import get_axon_ntff_profile_hook  # noqa: F401
        return
    except ImportError:
        pass
    from trn_agent_boot.trn_boot import _ntff_profile_via_ctypes
    hook = _ntff_profile_via_ctypes("/opt/axon/libaxon_pjrt.so")
    mod = types.ModuleType("antenv.axon_hooks")
    mod.get_axon_ntff_profile_hook = lambda: hook
    mod.set_axon_ntff_profile_hook = lambda h: None
    sys.modules["antenv.axon_hooks"] = mod


def _run(inputs, trace=False):
    from concourse import bass_utils

    if trace:
        _install_ntff_hook()

    x, y, bias = inputs["x"], inputs["y"], inputs["bias"]
    if np.count_nonzero(np.asarray(bias)):
        raise NotImplementedError("nonzero attention bias not supported")
    nc = _get_nc()
    maps = _in_maps(np.asarray(x, np.float32), np.asarray(y, np.float32),
                    np.asarray(inputs["Wq"], np.float32),
                    np.asarray(inputs["Wk"], np.float32),
                    np.asarray(inputs["Wv"], np.float32),
                    np.asarray(inputs["Wo"], np.float32))
    res = bass_utils.run_bass_kernel_spmd(
        nc, maps, list(range(8)), trace=trace)
    out = np.zeros((B, L, H), np.float32)
    for core in range(8):
        out[core // GP] += np.asarray(res.results[core]["out"], np.float32)
    return out, res


def kernel(**inputs):
    out, _ = _run(inputs, trace=False)
    return out
